# revision 11
# baseline (speedup 1.0000x reference)
"""Trainium2 Bass kernel for nn_DecoderVCSC (8-core SPMD).

Reference computation:
    c = z @ Wc.T + bc                                  (B, G)
    px_sigma = exp(-(c[:,iu] - c[:,jv]))               (B, P)   P = G*(G-1)/2
    h = px_sigma @ W1.T + b1                           (B, H)
    BN(training stats) + ReLU
    px_dropout = h @ Wdrop.T + bdrop
    px_shape   = h @ Wshape.T + bshape
    px_scale   = softmax(h @ Wscale.T + bscale)
    returns (px_shape, px_scale, px_dropout, px_sigma)

Sharding: the i<j pair blocks (block i = pairs (i, i+1..G-1)) are dealt
round-robin: core r handles blocks i = 8k + r, k = 0..127.  To keep the
SPMD instruction stream identical across cores, slot k is padded to
L_k = G-1-8k columns on every core (core r's true block length is
G-1-8k-r; the r pad columns are garbage on device and are (a) skipped by
the host unshard and (b) multiplied by host-zeroed W1T rows in the h
contraction).  The per-core shift r is absorbed on the host by passing a
rotated Wc (WcT_rot[:, t] = Wc[t+r, :]), so the device computes
cshift[b, t] = c[b, t+r] with core-independent addressing.

h is accumulated per-core over its p-slice and AllReduced across the 8
cores on-device; BN + heads + softmax run (redundantly) on every core.
b1 provably cancels in train-stats BN and is not sent to the device.
"""

import numpy as np

G = 1024
NIN = 64
H = 128
B = 256
P = G * (G - 1) // 2  # 523776
NCORES = 8
NSLOT = G // NCORES  # 128 slots per core
ALPHA = 1.0
BN_EPS = 1e-3

# Padded per-slot segment lengths/offsets (identical on all cores).
SEG_LEN = [G - 1 - NCORES * k for k in range(NSLOT)]  # 1023 - 8k
_off = np.cumsum([0] + SEG_LEN)
SEG_OFF = [int(x) for x in _off]
PCPAD = SEG_OFF[-1]  # 65920
NPT = PCPAD // 128  # 515 p-tiles of 128
assert PCPAD % 128 == 0

CHUNK = 2048  # sigma free-dim chunk width (elements)
CHUNKS = []
_c0 = 0
while _c0 < PCPAD:
    CHUNKS.append((_c0, min(_c0 + CHUNK, PCPAD)))
    _c0 += CHUNK


def _block_off(i):
    # global p offset of block i: sum_{t<i} (G-1-t)
    return i * (G - 1) - (i * (i - 1)) // 2


def _segments_in(lo, hi):
    """Yield (k, seg_lo, seg_hi) covering [lo,hi) split by slot boundaries."""
    out = []
    for k in range(NSLOT):
        s, e = SEG_OFF[k], SEG_OFF[k + 1]
        a, b = max(s, lo), min(e, hi)
        if a < b:
            out.append((k, a, b))
    return out


_NC_CACHE = {}


def _build_nc():
    if "nc" in _NC_CACHE:
        return _NC_CACHE["nc"]
    import concourse.bass as bass
    import concourse.mybir as mybir
    import concourse.tile as tile
    from concourse import bacc
    from concourse.masks import make_identity

    f32 = mybir.dt.float32
    f32r = mybir.dt.float32r
    AF = mybir.ActivationFunctionType
    ALU = mybir.AluOpType
    AX = mybir.AxisListType

    nc = bacc.Bacc(
        "TRN2", target_bir_lowering=False, debug=False, num_devices=NCORES
    )

    _pools = {}

    def _sgl(tc, shape, dtype, name, space="SBUF"):
        pool = _pools["psum" if space == "PSUM" else "const"]
        return pool.tile(shape, dtype, name=name, tag=name)

    # ---- I/O ----
    zT_d = nc.dram_tensor("zT", [NIN, B], f32, kind="ExternalInput").ap()
    wcT_d = nc.dram_tensor("wcT_rot", [NIN, G], f32, kind="ExternalInput").ap()
    bc_d = nc.dram_tensor("bc_rot", [1, G], f32, kind="ExternalInput").ap()
    w1t_d = nc.dram_tensor("w1t", [PCPAD, H], f32r, kind="ExternalInput").ap()
    gamma_d = nc.dram_tensor("gamma", [H, 1], f32, kind="ExternalInput").ap()
    beta_d = nc.dram_tensor("beta", [H, 1], f32, kind="ExternalInput").ap()
    wscT_d = nc.dram_tensor("wscT", [H, G], f32, kind="ExternalInput").ap()
    wshT_d = nc.dram_tensor("wshT", [H, G], f32, kind="ExternalInput").ap()
    wdrT_d = nc.dram_tensor("wdrT", [H, G], f32, kind="ExternalInput").ap()
    bsc_d = nc.dram_tensor("bsc", [1, G], f32, kind="ExternalInput").ap()
    bsh_d = nc.dram_tensor("bsh", [1, G], f32, kind="ExternalInput").ap()
    bdr_d = nc.dram_tensor("bdr", [1, G], f32, kind="ExternalInput").ap()

    psig_d = nc.dram_tensor("px_sig", [B, PCPAD], f32, kind="ExternalOutput").ap()
    psh_d = nc.dram_tensor("px_shape", [B, G], f32, kind="ExternalOutput").ap()
    psc_d = nc.dram_tensor("px_scale", [B, G], f32, kind="ExternalOutput").ap()
    pdr_d = nc.dram_tensor("px_drop", [B, G], f32, kind="ExternalOutput").ap()

    from contextlib import ExitStack

    with tile.TileContext(nc) as tc, ExitStack() as _stack:
        _pools["const"] = _stack.enter_context(tc.tile_pool(name="const", bufs=1))
        _pools["psum"] = _stack.enter_context(
            tc.tile_pool(name="constps", bufs=1, space="PSUM")
        )
        # ---------- constants ----------
        ident = _sgl(tc, [128, 128], f32, name="ident")
        make_identity(nc, ident[:])
        ones1f = _sgl(tc, [128, 128], f32, name="ones1f")
        nc.vector.memset(ones1f[:], 1.0)
        ones1 = ones1f[0:1, :]

        # ---------- small input loads ----------
        zT_sbf = _sgl(tc, [128, B], f32, name="zT_sbf")
        nc.sync.dma_start(zT_sbf[0:NIN, :], zT_d[:])
        zT_sb = zT_sbf[0:NIN, :]
        wcT_sbf = _sgl(tc, [128, G], f32, name="wcT_sbf")
        nc.sync.dma_start(wcT_sbf[0:NIN, :], wcT_d[:])
        wcT_sb = wcT_sbf[0:NIN, :]
        bc_sbf = _sgl(tc, [128, G], f32, name="bc_sbf")
        nc.sync.dma_start(bc_sbf[0:1, :], bc_d[:])
        bc_sb = bc_sbf[0:1, :]
        gamma_sb = _sgl(tc, [H, 1], f32, name="gamma_sb")
        nc.sync.dma_start(gamma_sb[:], gamma_d[:])
        beta_sb = _sgl(tc, [H, 1], f32, name="beta_sb")
        nc.sync.dma_start(beta_sb[:], beta_d[:])
        hw_sb = _sgl(tc, [H, 3 * G], f32, name="hw_sb")  # head weights (T)
        nc.sync.dma_start(hw_sb[:, 0:G], wshT_d[:])
        nc.sync.dma_start(hw_sb[:, G : 2 * G], wscT_d[:])
        nc.sync.dma_start(hw_sb[:, 2 * G : 3 * G], wdrT_d[:])
        hb_sbf = _sgl(tc, [128, 3 * G], f32, name="hb_sbf")  # head biases
        hb_sb = hb_sbf[0:1, :]
        nc.sync.dma_start(hb_sbf[0:1, 0:G], bsh_d[:])
        nc.sync.dma_start(hb_sbf[0:1, G : 2 * G], bsc_d[:])
        nc.sync.dma_start(hb_sbf[0:1, 2 * G : 3 * G], bdr_d[:])

        # ---------- cshift = (z @ WcT_rot) + bc_rot,  [2x128 b, 1024 g'] ----------
        csh = []  # per b-tile SBUF [128, G]
        ncsh = []
        with tc.tile_pool(name="cpsum", bufs=2, space="PSUM") as cps_pool:
            for bt in range(2):
                c_sb = _sgl(tc, [128, G], f32, name=f"csh{bt}")
                n_sb = _sgl(tc, [128, G], f32, name=f"ncsh{bt}")
                for gh in range(2):  # two 512-wide psum tiles
                    cps = cps_pool.tile([128, 512], f32, tag="cps", name="cps")
                    gsl = slice(gh * 512, (gh + 1) * 512)
                    # bias broadcast via rank-1 matmul, then accumulate z@WcT
                    nc.tensor.matmul(
                        cps[:], ones1[:, 0:128], bc_sb[:, gsl], start=True, stop=False
                    )
                    nc.tensor.matmul(
                        cps[:],
                        zT_sb[:, bt * 128 : (bt + 1) * 128],
                        wcT_sb[:, gsl],
                        start=False,
                        stop=True,
                    )
                    nc.scalar.copy(c_sb[:, gsl], cps[:])
                    nc.scalar.mul(n_sb[:, gsl], cps[:], -1.0)
                csh.append(c_sb)
                ncsh.append(n_sb)

        # ---------- main loop: sigma gen -> HBM + transpose -> h matmul ----------
        h_ps = _sgl(tc, [128, B], f32, name="h_ps", space="PSUM")  # h.T accumulator
        pt_global = 0
        with (
            tc.tile_pool(name="sig", bufs=2) as sig_pool,
            tc.tile_pool(name="sigT", bufs=6) as sigT_pool,
            tc.tile_pool(name="sigTps", bufs=4, space="PSUM") as sigTps_pool,
            tc.tile_pool(name="w1", bufs=2) as w1_pool,
        ):
            for (c0, c1) in CHUNKS:
                W = c1 - c0
                nt = W // 128
                # -- sigma generation (ACT), both b-tiles --
                sig = [
                    sig_pool.tile([128, W], f32, tag=f"sig{bt}", name=f"sig{bt}") for bt in range(2)
                ]
                for (k, a, b) in _segments_in(c0, c1):
                    # sigma[:, a:b] = exp(cshift[:, 8k+1+(a-S_k) : ...] - cshift[:, 8k])
                    j0 = NCORES * k + 1 + (a - SEG_OFF[k])
                    for bt in range(2):
                        nc.scalar.activation(
                            sig[bt][:, a - c0 : b - c0],
                            csh[bt][:, j0 : j0 + (b - a)],
                            AF.Exp,
                            bias=ncsh[bt][:, NCORES * k : NCORES * k + 1],
                            scale=1.0,
                        )
                # -- write sigma chunk to HBM --
                for bt in range(2):
                    nc.sync.dma_start(
                        psig_d[bt * 128 : (bt + 1) * 128, c0:c1], sig[bt][:]
                    )
                # -- W1T chunk load: [W, H] rows -> SBUF [128, nt*H] --
                w1c = w1_pool.tile([128, nt * H], f32r, tag="w1c", name="w1c")
                nc.sync.dma_start(
                    w1c[:].rearrange("p (t h) -> p t h", h=H),
                    w1t_d[c0:c1, :].rearrange("(t p) h -> p t h", p=128),
                )
                # -- per p-tile: transpose sigma, accumulate h --
                for t in range(nt):
                    tps = sigTps_pool.tile([128, B], f32, tag="tps", name="tps")
                    for bt in range(2):
                        nc.tensor.transpose(
                            tps[:, bt * 128 : (bt + 1) * 128],
                            sig[bt][:, t * 128 : (t + 1) * 128],
                            ident[:],
                        )
                    sT = sigT_pool.tile([128, B], f32r, tag="sT", name="sT")
                    nc.vector.tensor_copy(sT[:], tps[:])
                    nc.tensor.matmul(
                        h_ps[:],
                        w1c[:, t * H : (t + 1) * H],
                        sT[:],
                        start=(pt_global == 0),
                        stop=(pt_global == NPT - 1),
                        skip_group_check=True,
                    )
                    pt_global += 1

        # ---------- AllReduce h across cores ----------
        h_sb = _sgl(tc, [128, B], f32, name="h_sb")
        nc.scalar.copy(h_sb[:], h_ps[:])
        with tc.tile_pool(name="dram", bufs=2, space="DRAM") as dram:
            h_in = dram.tile([128, B], f32, name="hbounce")
            h_out = dram.tile([128, B], f32, name="hbounce")
            nc.gpsimd.dma_start(h_in[:], h_sb[:])
            nc.gpsimd.collective_compute(
                "AllReduce",
                ALU.add,
                replica_groups=[list(range(NCORES))],
                ins=[h_in[:].opt()],
                outs=[h_out[:].opt()],
            )
            hall = _sgl(tc, [128, B], f32, name="hall")
            nc.gpsimd.dma_start(hall[:], h_out[:])

        # ---------- BatchNorm (training stats) + ReLU, in [k, b] layout ----------
        musum = _sgl(tc, [128, 1], f32, name="musum")
        nc.vector.tensor_reduce(musum[:], hall[:], axis=AX.X, op=ALU.add)
        hsq = _sgl(tc, [128, B], f32, name="hsq")
        nc.scalar.activation(hsq[:], hall[:], AF.Square)
        sqsum = _sgl(tc, [128, 1], f32, name="sqsum")
        nc.vector.tensor_reduce(sqsum[:], hsq[:], axis=AX.X, op=ALU.add)
        mu = _sgl(tc, [128, 1], f32, name="mu")
        nc.vector.tensor_scalar_mul(mu[:], musum[:], 1.0 / B)
        ex2 = _sgl(tc, [128, 1], f32, name="ex2")
        nc.vector.tensor_scalar_mul(ex2[:], sqsum[:], 1.0 / B)
        musq = _sgl(tc, [128, 1], f32, name="musq")
        nc.vector.tensor_tensor(musq[:], mu[:], mu[:], op=ALU.mult)
        var = _sgl(tc, [128, 1], f32, name="var")
        nc.vector.tensor_tensor(var[:], ex2[:], musq[:], op=ALU.subtract)
        varp = _sgl(tc, [128, 1], f32, name="varp")
        nc.vector.tensor_scalar_add(varp[:], var[:], BN_EPS)
        vinv = _sgl(tc, [128, 1], f32, name="vinv")
        nc.vector.reciprocal(vinv[:], varp[:])
        rst = _sgl(tc, [128, 1], f32, name="rst")
        nc.scalar.sqrt(rst[:], vinv[:])
        bnsc = _sgl(tc, [128, 1], f32, name="bnsc")
        nc.vector.tensor_tensor(bnsc[:], gamma_sb[:], rst[:], op=ALU.mult)
        mbs = _sgl(tc, [128, 1], f32, name="mbs")
        nc.vector.tensor_tensor(mbs[:], mu[:], bnsc[:], op=ALU.mult)
        bnbi = _sgl(tc, [128, 1], f32, name="bnbi")
        nc.vector.tensor_tensor(bnbi[:], beta_sb[:], mbs[:], op=ALU.subtract)
        hbn = _sgl(tc, [128, B], f32, name="hbn")
        nc.scalar.activation(hbn[:], hall[:], AF.Relu, bias=bnbi[:], scale=bnsc[:])

        # ---------- heads: out[b, g] = hbn.T @ WxT + bx ----------
        # head order in hw_sb/hb_sb: 0=shape, 1=scale, 2=drop
        head_out_d = [psh_d, psc_d, pdr_d]
        with (
            tc.tile_pool(name="hps", bufs=4, space="PSUM") as hps_pool,
            tc.tile_pool(name="hout", bufs=4) as hout_pool,
        ):
            for hd in range(3):
                for bt in range(2):
                    o_sb = hout_pool.tile([128, G], f32, tag="o_sb", name="o_sb")
                    for gh in range(2):
                        hps = hps_pool.tile([128, 512], f32, tag="hps", name="hps")
                        gsl = slice(hd * G + gh * 512, hd * G + (gh + 1) * 512)
                        nc.tensor.matmul(
                            hps[:], ones1[:, 0:128], hb_sb[:, gsl],
                            start=True, stop=False,
                        )
                        nc.tensor.matmul(
                            hps[:],
                            hbn[:, bt * 128 : (bt + 1) * 128],
                            hw_sb[:, gsl],
                            start=False,
                            stop=True,
                        )
                        nc.scalar.copy(o_sb[:, gh * 512 : (gh + 1) * 512], hps[:])
                    rsl = slice(bt * 128, (bt + 1) * 128)
                    if hd == 1:  # scale head: softmax over g
                        mx = hout_pool.tile([128, 1], f32, tag="mx", name="mx")
                        nc.vector.tensor_reduce(mx[:], o_sb[:], axis=AX.X, op=ALU.max)
                        nmx = hout_pool.tile([128, 1], f32, tag="nmx", name="nmx")
                        nc.vector.tensor_scalar_mul(nmx[:], mx[:], -1.0)
                        exl = hout_pool.tile([128, G], f32, tag="exl", name="exl")
                        nc.scalar.activation(exl[:], o_sb[:], AF.Exp, bias=nmx[:])
                        ssum = hout_pool.tile([128, 1], f32, tag="ssum", name="ssum")
                        nc.vector.tensor_reduce(
                            ssum[:], exl[:], axis=AX.X, op=ALU.add
                        )
                        sinv = hout_pool.tile([128, 1], f32, tag="sinv", name="sinv")
                        nc.vector.reciprocal(sinv[:], ssum[:])
                        smx = hout_pool.tile([128, G], f32, tag="smx", name="smx")
                        nc.vector.tensor_scalar_mul(smx[:], exl[:], sinv[:])
                        nc.sync.dma_start(head_out_d[hd][rsl, :], smx[:])
                    else:
                        nc.sync.dma_start(head_out_d[hd][rsl, :], o_sb[:])

    nc.compile()
    _NC_CACHE["nc"] = nc
    return nc


_HOST_CACHE = {}


def _host_maps():
    """Static per-core index maps (depend only on shapes)."""
    if "maps" in _HOST_CACHE:
        return _HOST_CACHE["maps"]
    # per core r, per slot k: true i = 8k+r, valid len = G-1-i, local seg at SEG_OFF[k]
    maps = []
    for r in range(NCORES):
        rows = []  # (local_lo, local_hi, global_lo, global_hi)
        for k in range(NSLOT):
            i = NCORES * k + r
            vlen = G - 1 - i
            if vlen <= 0:
                continue
            o = _block_off(i)
            rows.append((SEG_OFF[k], SEG_OFF[k] + vlen, o, o + vlen))
        maps.append(rows)
    _HOST_CACHE["maps"] = maps
    return maps


def kernel(z, Wc, bc, W1, b1, gamma, beta, Wscale, bscale,
           Wshape, bshape, Wdrop, bdrop):
    return _run(z, Wc, bc, W1, b1, gamma, beta, Wscale, bscale,
                Wshape, bshape, Wdrop, bdrop)[0]


def _prep_in_maps(z, Wc, bc, W1, b1, gamma, beta, Wscale, bscale,
                  Wshape, bshape, Wdrop, bdrop):
    f32 = np.float32
    z = np.asarray(z, f32)
    Wc = np.asarray(Wc, f32)
    bc = np.asarray(bc, f32)
    W1 = np.asarray(W1, f32)
    gamma = np.asarray(gamma, f32)
    beta = np.asarray(beta, f32)

    nc = _build_nc()
    maps = _host_maps()

    zT = np.ascontiguousarray(z.T)  # [NIN, B]
    WcT = np.ascontiguousarray(Wc.T)  # [NIN, G]
    W1T = np.ascontiguousarray(W1.T)  # [P, H]
    shared = {
        "zT": zT,
        "gamma": np.ascontiguousarray(gamma.reshape(H, 1)),
        "beta": np.ascontiguousarray(beta.reshape(H, 1)),
        "wscT": np.ascontiguousarray(np.asarray(Wscale, f32).T),
        "wshT": np.ascontiguousarray(np.asarray(Wshape, f32).T),
        "wdrT": np.ascontiguousarray(np.asarray(Wdrop, f32).T),
        "bsc": np.ascontiguousarray(np.asarray(bscale, f32).reshape(1, G)),
        "bsh": np.ascontiguousarray(np.asarray(bshape, f32).reshape(1, G)),
        "bdr": np.ascontiguousarray(np.asarray(bdrop, f32).reshape(1, G)),
    }

    in_maps = []
    for r in range(NCORES):
        wcT_rot = np.zeros((NIN, G), f32)
        wcT_rot[:, : G - r] = WcT[:, r:]
        bc_rot = np.zeros((1, G), f32)
        bc_rot[0, : G - r] = bc[r:]
        w1t_r = np.zeros((PCPAD, H), f32)
        for (ll, lh, gl, gh) in maps[r]:
            w1t_r[ll:lh] = W1T[gl:gh]
        m = dict(shared)
        m["wcT_rot"] = wcT_rot
        m["bc_rot"] = bc_rot
        m["w1t"] = w1t_r
        in_maps.append(m)
    return in_maps


def _run(z, Wc, bc, W1, b1, gamma, beta, Wscale, bscale,
         Wshape, bshape, Wdrop, bdrop, trace=False, trace_cores=None):
    from concourse import bass_utils

    f32 = np.float32
    nc = _build_nc()
    maps = _host_maps()
    in_maps = _prep_in_maps(z, Wc, bc, W1, b1, gamma, beta, Wscale, bscale,
                            Wshape, bshape, Wdrop, bdrop)
    res = bass_utils.run_bass_kernel_spmd(
        nc, in_maps, core_ids=list(range(NCORES)),
        trace=trace, trace_cores=trace_cores,
    )
    outs = res.results

    px_sigma = np.empty((B, P), f32)
    for r in range(NCORES):
        sig_r = outs[r]["px_sig"]
        for (ll, lh, gl, gh) in maps[r]:
            px_sigma[:, gl:gh] = sig_r[:, ll:lh]

    px_shape = outs[0]["px_shape"]
    px_scale = outs[0]["px_scale"]
    px_dropout = outs[0]["px_drop"]
    return (px_shape, px_scale, px_dropout, px_sigma), res


# revision 33
# speedup vs baseline: 761.1103x; 761.1103x over previous
"""Trainium2 Bass kernel for nn_DecoderVCSC (8-core SPMD).

Reference computation:
    c = z @ Wc.T + bc                                  (B, G)
    px_sigma = exp(-(c[:,iu] - c[:,jv]))               (B, P)   P = G*(G-1)/2
    h = px_sigma @ W1.T + b1                           (B, H)
    BN(training stats) + ReLU
    px_dropout = h @ Wdrop.T + bdrop
    px_shape   = h @ Wshape.T + bshape
    px_scale   = softmax(h @ Wscale.T + bscale)
    returns (px_shape, px_scale, px_dropout, px_sigma)

Sharding: the i<j pair blocks (block i = pairs (i, i+1..G-1)) are dealt
round-robin: core r handles blocks i = 8k + r, k = 0..127.  To keep the
SPMD instruction stream identical across cores, slot k is padded to
L_k = G-1-8k columns on every core (core r's true block length is
G-1-8k-r; the r pad columns are garbage on device and are (a) skipped by
the host unshard and (b) multiplied by host-zeroed W1T rows in the h
contraction).  The per-core shift r is absorbed on the host by passing a
rotated Wc (WcT_rot[:, t] = Wc[t+r, :]), so the device computes
cshift[b, t] = c[b, t+r] with core-independent addressing.

h is accumulated per-core over its p-slice and AllReduced across the 8
cores on-device; BN + heads + softmax run (redundantly) on every core.
b1 provably cancels in train-stats BN and is not sent to the device.
"""

import numpy as np

G = 1024
NIN = 64
H = 128
B = 256
P = G * (G - 1) // 2  # 523776
NCORES = 8
NSLOT = G // NCORES  # 128 slots per core
ALPHA = 1.0
BN_EPS = 1e-3

# Padded per-slot segment lengths/offsets (identical on all cores).
SEG_LEN = [G - 1 - NCORES * k for k in range(NSLOT)]  # 1023 - 8k
_off = np.cumsum([0] + SEG_LEN)
SEG_OFF = [int(x) for x in _off]
PCPAD = SEG_OFF[-1]  # 65920
NPT = PCPAD // 128  # 515 p-tiles of 128
assert PCPAD % 128 == 0

CHUNK = 2048  # sigma free-dim chunk width (elements)
CHUNKS = []
_c0 = 0
while _c0 < PCPAD:
    CHUNKS.append((_c0, min(_c0 + CHUNK, PCPAD)))
    _c0 += CHUNK


def _block_off(i):
    # global p offset of block i: sum_{t<i} (G-1-t)
    return i * (G - 1) - (i * (i - 1)) // 2


def _segments_in(lo, hi):
    """Yield (k, seg_lo, seg_hi) covering [lo,hi) split by slot boundaries."""
    out = []
    for k in range(NSLOT):
        s, e = SEG_OFF[k], SEG_OFF[k + 1]
        a, b = max(s, lo), min(e, hi)
        if a < b:
            out.append((k, a, b))
    return out


_NC_CACHE = {}


def _build_nc(single=False):
    key = "nc1" if single else "nc"
    if key in _NC_CACHE:
        return _NC_CACHE[key]
    import concourse.bass as bass
    import concourse.mybir as mybir
    import concourse.tile as tile
    from concourse import bacc
    from concourse.masks import make_identity

    f32 = mybir.dt.float32
    f32r = mybir.dt.float32r
    f16 = mybir.dt.float16
    AF = mybir.ActivationFunctionType
    ALU = mybir.AluOpType
    AX = mybir.AxisListType

    nc = bacc.Bacc(
        "TRN2", target_bir_lowering=False, debug=False,
        num_devices=1 if single else NCORES,
    )

    _pools = {}

    def _sgl(tc, shape, dtype, name, space="SBUF"):
        pool = _pools["psum" if space == "PSUM" else "const"]
        return pool.tile(shape, dtype, name=name, tag=name)

    # ---- I/O ----
    zT_d = nc.dram_tensor("zT", [NIN, B], f32, kind="ExternalInput").ap()
    wcT_d = nc.dram_tensor("wcT_rot", [NIN, G], f32, kind="ExternalInput").ap()
    bc_d = nc.dram_tensor("bc_rot", [1, G], f32, kind="ExternalInput").ap()
    w1t_d = nc.dram_tensor("w1t", [128, NPT * H], f16, kind="ExternalInput").ap()
    gamma_d = nc.dram_tensor("gamma", [H, 1], f32, kind="ExternalInput").ap()
    beta_d = nc.dram_tensor("beta", [H, 1], f32, kind="ExternalInput").ap()
    GS = G // NCORES  # per-core head-output columns
    wscT_d = nc.dram_tensor("wscT", [H, G], f32r, kind="ExternalInput").ap()
    wshT_d = nc.dram_tensor("wshT", [H, GS], f32r, kind="ExternalInput").ap()
    wdrT_d = nc.dram_tensor("wdrT", [H, GS], f32r, kind="ExternalInput").ap()
    bsc_d = nc.dram_tensor("bsc", [1, G], f32r, kind="ExternalInput").ap()
    bsh_d = nc.dram_tensor("bsh", [1, GS], f32r, kind="ExternalInput").ap()
    bdr_d = nc.dram_tensor("bdr", [1, GS], f32r, kind="ExternalInput").ap()

    psig_d = nc.dram_tensor("px_sig", [B, PCPAD], f16, kind="ExternalOutput").ap()
    psh_d = nc.dram_tensor("px_shape", [B, GS], f32, kind="ExternalOutput").ap()
    psc_d = nc.dram_tensor("px_scale", [B, G], f32, kind="ExternalOutput").ap()
    pdr_d = nc.dram_tensor("px_drop", [B, GS], f32, kind="ExternalOutput").ap()

    from contextlib import ExitStack

    with tile.TileContext(nc) as tc, ExitStack() as _stack:
        _pools["const"] = _stack.enter_context(tc.tile_pool(name="const", bufs=1))
        _pools["psum"] = _stack.enter_context(
            tc.tile_pool(name="constps", bufs=1, space="PSUM")
        )
        # ---------- constants ----------
        identF = _sgl(tc, [128, 128], f32, name="identF")
        make_identity(nc, identF[:])
        ident = _sgl(tc, [128, 128], f16, name="ident")
        nc.vector.tensor_copy(ident[:], identF[:])
        identr = ident[:]
        ones1cf = _sgl(tc, [128, 128], f32, name="ones1cf")
        nc.vector.memset(ones1cf[:], 1.0)
        ones1c = ones1cf[0:1, :]
        ones1f = _sgl(tc, [128, 128], f32r, name="ones1f")
        nc.vector.tensor_copy(ones1f[:], ones1cf[:])
        ones1 = ones1f[0:1, :]

        # ---------- small input loads ----------
        zT_sbf = _sgl(tc, [128, B], f32, name="zT_sbf")
        nc.sync.dma_start(zT_sbf[0:NIN, :], zT_d[:])
        zT_sb = zT_sbf[0:NIN, :]
        wcT_sbf = _sgl(tc, [128, G], f32, name="wcT_sbf")
        nc.sync.dma_start(wcT_sbf[0:NIN, :], wcT_d[:])
        wcT_sb = wcT_sbf[0:NIN, :]
        bc_sbf = _sgl(tc, [128, G], f32, name="bc_sbf")
        nc.sync.dma_start(bc_sbf[0:1, :], bc_d[:])
        bc_sb = bc_sbf[0:1, :]
        gamma_sb = _sgl(tc, [H, 1], f32, name="gamma_sb")
        nc.sync.dma_start(gamma_sb[:], gamma_d[:])
        beta_sb = _sgl(tc, [H, 1], f32, name="beta_sb")
        nc.sync.dma_start(beta_sb[:], beta_d[:])
        hw_sb = _sgl(tc, [H, G + 2 * GS], f32r, name="hw_sb")  # head weights (T)
        nc.sync.dma_start(hw_sb[:, 0:GS], wshT_d[:])
        nc.sync.dma_start(hw_sb[:, GS : 2 * GS], wdrT_d[:])
        nc.sync.dma_start(hw_sb[:, 2 * GS : 2 * GS + G], wscT_d[:])
        hb_sbf = _sgl(tc, [128, G + 2 * GS], f32r, name="hb_sbf")  # head biases
        hb_sb = hb_sbf[0:1, :]
        nc.sync.dma_start(hb_sbf[0:1, 0:GS], bsh_d[:])
        nc.sync.dma_start(hb_sbf[0:1, GS : 2 * GS], bdr_d[:])
        nc.sync.dma_start(hb_sbf[0:1, 2 * GS : 2 * GS + G], bsc_d[:])

        # ---------- cshift = (z @ WcT_rot) + bc_rot,  [2x128 b, 1024 g'] ----------
        csh = []  # per b-tile SBUF [128, G]
        ebuf = []  # exp(cshift)
        fbuf = []  # exp(-cshift)
        with tc.tile_pool(name="cpsum", bufs=4, space="PSUM") as cps_pool:
            for bt in range(2):
                c_sb = _sgl(tc, [128, G], f32, name=f"csh{bt}")
                e_sb = _sgl(tc, [128, G], f32, name=f"ebuf{bt}")
                f_sb = _sgl(tc, [128, G], f32, name=f"fbuf{bt}")
                for gh in range(2):  # two 512-wide psum tiles
                    cps = cps_pool.tile([128, 512], f32, tag="cps", name="cps")
                    gsl = slice(gh * 512, (gh + 1) * 512)
                    # bias broadcast via rank-1 matmul, then accumulate z@WcT
                    nc.tensor.matmul(
                        cps[:], ones1c[:, 0:128], bc_sb[:, gsl], start=True, stop=False
                    )
                    nc.tensor.matmul(
                        cps[:],
                        zT_sb[:, bt * 128 : (bt + 1) * 128],
                        wcT_sb[:, gsl],
                        start=False,
                        stop=True,
                    )
                    nc.scalar.copy(c_sb[:, gsl], cps[:])
                    nc.scalar.activation(e_sb[:, gsl], cps[:], AF.Exp)
                    nc.scalar.activation(f_sb[:, gsl], cps[:], AF.Exp, scale=-1.0)
                csh.append(c_sb)
                ebuf.append(e_sb)
                fbuf.append(f_sb)

        # ---------- main loop: sigma gen -> HBM + transpose -> h matmul ----------
        h_ps = _sgl(tc, [128, B], f32, name="h_ps", space="PSUM")  # h.T accumulator
        pt_global = 0
        with (
            tc.tile_pool(name="sig", bufs=3) as sig_pool,
            tc.tile_pool(name="sigT", bufs=4) as sigT_pool,
            tc.tile_pool(name="sigTps", bufs=4, space="PSUM") as sigTps_pool,
            tc.tile_pool(name="w1", bufs=12) as w1_pool,
        ):
            for (c0, c1) in CHUNKS:
                W = c1 - c0
                nt = W // 128
                # -- sigma generation (ACT), both b-tiles --
                sig = [
                    sig_pool.tile([128, W], f16, tag=f"sig{bt}", name=f"sig{bt}") for bt in range(2)
                ]
                for (k, a, b) in _segments_in(c0, c1):
                    # sigma[:, a:b] = exp(cshift[:, 8k+1+(a-S_k) : ...] - cshift[:, 8k])
                    j0 = NCORES * k + 1 + (a - SEG_OFF[k])
                    for bt in range(2):
                        nc.vector.tensor_scalar_mul(
                            sig[bt][:, a - c0 : b - c0],
                            ebuf[bt][:, j0 : j0 + (b - a)],
                            fbuf[bt][:, NCORES * k : NCORES * k + 1],
                        )
                # -- write sigma chunk to HBM --
                for bt in range(2):
                    nc.sync.dma_start(
                        psig_d[bt * 128 : (bt + 1) * 128, c0:c1], sig[bt][:]
                    )
                # -- W1T chunk load: [W, H] rows -> SBUF [128, nt*H] --
                w1c = w1_pool.tile([128, nt * H], f16, tag="w1c", name="w1c")
                nc.gpsimd.dma_start(
                    w1c[:], w1t_d[:, (c0 // 128) * H : (c1 // 128) * H]
                )
                # -- per p-tile: transpose sigma, accumulate h --
                for t2 in range(0, nt, 2):
                    npair = min(2, nt - t2)
                    tps = sigTps_pool.tile([128, 512], f16, tag="tps", name="tps")
                    for ti in range(npair):
                        for bt in range(2):
                            nc.tensor.transpose(
                                tps[:, ti * 256 + bt * 128 : ti * 256 + (bt + 1) * 128],
                                sig[bt][:, (t2 + ti) * 128 : (t2 + ti + 1) * 128],
                                identr[:],
                            )
                    sT = sigT_pool.tile([128, 512], f16, tag="sT", name="sT")
                    if (t2 // 2) % 5 == 0:
                        nc.vector.tensor_copy(
                            sT[:, : npair * 256], tps[:, : npair * 256]
                        )
                    else:
                        nc.scalar.copy(
                            sT[:, : npair * 256], tps[:, : npair * 256]
                        )
                    for ti in range(npair):
                        t = t2 + ti
                        nc.tensor.matmul(
                            h_ps[:],
                            w1c[:, t * H : (t + 1) * H],
                            sT[:, ti * 256 : (ti + 1) * 256],
                            start=(pt_global == 0),
                            stop=(pt_global == NPT - 1),
                            skip_group_check=True,
                        )
                        pt_global += 1

        # ---------- AllReduce h across cores ----------
        h_sb = _sgl(tc, [128, B], f32, name="h_sb")
        nc.scalar.copy(h_sb[:], h_ps[:])
        with tc.tile_pool(name="dram", bufs=2, space="DRAM") as dram:
            h_in = dram.tile([128, B], f32, name="hbounce")
            h_out = dram.tile([128, B], f32, name="hbounce")
            nc.gpsimd.dma_start(h_in[:], h_sb[:])
            if single:
                # cost-model variant: stand in for the AllReduce with a copy
                nc.gpsimd.dma_start(h_out[:], h_in[:])
            else:
                nc.gpsimd.collective_compute(
                    "AllReduce",
                    ALU.add,
                    replica_groups=[list(range(NCORES))],
                    ins=[h_in[:].opt()],
                    outs=[h_out[:].opt()],
                )
            hall = _sgl(tc, [128, B], f32, name="hall")
            nc.gpsimd.dma_start(hall[:], h_out[:])

        # ---------- BatchNorm (training stats) + ReLU, in [k, b] layout ----------
        musum = _sgl(tc, [128, 1], f32, name="musum")
        nc.vector.tensor_reduce(musum[:], hall[:], axis=AX.X, op=ALU.add)
        hsq = _sgl(tc, [128, B], f32, name="hsq")
        sqsum = _sgl(tc, [128, 1], f32, name="sqsum")
        nc.scalar.activation(hsq[:], hall[:], AF.Square, accum_out=sqsum[:])
        mu = _sgl(tc, [128, 1], f32, name="mu")
        nc.vector.tensor_scalar_mul(mu[:], musum[:], 1.0 / B)
        musq = _sgl(tc, [128, 1], f32, name="musq")
        nc.vector.tensor_tensor(musq[:], mu[:], mu[:], op=ALU.mult)
        var = _sgl(tc, [128, 1], f32, name="var")
        nc.vector.scalar_tensor_tensor(
            var[:], sqsum[:], 1.0 / B, musq[:],
            op0=ALU.mult, op1=ALU.subtract,
        )
        varp = _sgl(tc, [128, 1], f32, name="varp")
        nc.vector.tensor_scalar_add(varp[:], var[:], BN_EPS)
        lnv = _sgl(tc, [128, 1], f32, name="lnv")
        nc.scalar.activation(lnv[:], varp[:], AF.Ln)
        rst = _sgl(tc, [128, 1], f32, name="rst")
        nc.scalar.activation(rst[:], lnv[:], AF.Exp, scale=-0.5)
        bnsc = _sgl(tc, [128, 1], f32, name="bnsc")
        nc.vector.tensor_tensor(bnsc[:], gamma_sb[:], rst[:], op=ALU.mult)
        mbs = _sgl(tc, [128, 1], f32, name="mbs")
        nc.vector.tensor_tensor(mbs[:], mu[:], bnsc[:], op=ALU.mult)
        bnbi = _sgl(tc, [128, 1], f32, name="bnbi")
        nc.vector.tensor_tensor(bnbi[:], beta_sb[:], mbs[:], op=ALU.subtract)
        hbn = _sgl(tc, [128, B], f32r, name="hbn")
        nc.scalar.activation(hbn[:], hall[:], AF.Relu, bias=bnbi[:], scale=bnsc[:])

        # ---------- heads ----------
        # sharded shape/drop: this core computes its GS output columns
        with (
            tc.tile_pool(name="hps", bufs=3, space="PSUM") as hps_pool,
            tc.tile_pool(name="hout", bufs=4) as hout_pool,
        ):
            for hd, (out_d, ev_eng) in enumerate(
                [(psh_d, nc.scalar), (pdr_d, nc.vector)]
            ):
                off = hd * GS
                for bt in range(2):
                    hps = hps_pool.tile([128, 512], f32, tag="hps", name="hps")[:, 0:GS]
                    nc.tensor.matmul(
                        hps[:], ones1[:, 0:128], hb_sb[:, off : off + GS],
                        start=True, stop=False,
                    )
                    nc.tensor.matmul(
                        hps[:],
                        hbn[:, bt * 128 : (bt + 1) * 128],
                        hw_sb[:, off : off + GS],
                        start=False,
                        stop=True,
                    )
                    o_sb = hout_pool.tile([128, GS], f32, tag="osm", name="o_sb")
                    if hd == 0:
                        ev_eng.copy(o_sb[:], hps[:])
                    else:
                        ev_eng.tensor_copy(o_sb[:], hps[:])
                    nc.sync.dma_start(out_d[bt * 128 : (bt + 1) * 128, :], o_sb[:])
            # scale head: full G + softmax, replicated on every core
            for bt in range(2):
                o_sb = hout_pool.tile([128, G], f32, tag="o_sb", name="o_sb")
                for gh in range(2):
                    hps = hps_pool.tile([128, 512], f32, tag="hps", name="hps")
                    gsl = slice(2 * GS + gh * 512, 2 * GS + (gh + 1) * 512)
                    nc.tensor.matmul(
                        hps[:], ones1[:, 0:128], hb_sb[:, gsl],
                        start=True, stop=False,
                    )
                    nc.tensor.matmul(
                        hps[:],
                        hbn[:, bt * 128 : (bt + 1) * 128],
                        hw_sb[:, gsl],
                        start=False,
                        stop=True,
                    )
                    if gh == 0:
                        nc.scalar.copy(o_sb[:, gh * 512 : (gh + 1) * 512], hps[:])
                    else:
                        nc.vector.tensor_copy(
                            o_sb[:, gh * 512 : (gh + 1) * 512], hps[:]
                        )
                mx = hout_pool.tile([128, 1], f32, tag="mx", name="mx")
                nc.vector.tensor_reduce(mx[:], o_sb[:], axis=AX.X, op=ALU.max)
                nmx = hout_pool.tile([128, 1], f32, tag="nmx", name="nmx")
                nc.vector.tensor_scalar_mul(nmx[:], mx[:], -1.0)
                exl = hout_pool.tile([128, G], f32, tag="exl", name="exl")
                nc.scalar.activation(exl[:], o_sb[:], AF.Exp, bias=nmx[:])
                ssum = hout_pool.tile([128, 1], f32, tag="ssum", name="ssum")
                nc.vector.tensor_reduce(ssum[:], exl[:], axis=AX.X, op=ALU.add)
                sinv = hout_pool.tile([128, 1], f32, tag="sinv", name="sinv")
                nc.vector.reciprocal(sinv[:], ssum[:])
                smx = hout_pool.tile([128, G], f32, tag="smx", name="smx")
                nc.vector.tensor_scalar_mul(smx[:], exl[:], sinv[:])
                nc.sync.dma_start(psc_d[bt * 128 : (bt + 1) * 128, :], smx[:])

    nc.compile()
    _NC_CACHE[key] = nc
    return nc


_HOST_CACHE = {}


def _host_maps():
    """Static per-core index maps (depend only on shapes)."""
    if "maps" in _HOST_CACHE:
        return _HOST_CACHE["maps"]
    # per core r, per slot k: true i = 8k+r, valid len = G-1-i, local seg at SEG_OFF[k]
    maps = []
    for r in range(NCORES):
        rows = []  # (local_lo, local_hi, global_lo, global_hi)
        for k in range(NSLOT):
            i = NCORES * k + r
            vlen = G - 1 - i
            if vlen <= 0:
                continue
            o = _block_off(i)
            rows.append((SEG_OFF[k], SEG_OFF[k] + vlen, o, o + vlen))
        maps.append(rows)
    _HOST_CACHE["maps"] = maps
    return maps


def kernel(z, Wc, bc, W1, b1, gamma, beta, Wscale, bscale,
           Wshape, bshape, Wdrop, bdrop):
    return _run(z, Wc, bc, W1, b1, gamma, beta, Wscale, bscale,
                Wshape, bshape, Wdrop, bdrop)[0]


def _prep_in_maps(z, Wc, bc, W1, b1, gamma, beta, Wscale, bscale,
                  Wshape, bshape, Wdrop, bdrop):
    f32 = np.float32
    z = np.asarray(z, f32)
    Wc = np.asarray(Wc, f32)
    bc = np.asarray(bc, f32)
    W1 = np.asarray(W1, f32)
    gamma = np.asarray(gamma, f32)
    beta = np.asarray(beta, f32)
    maps = _host_maps()

    zT = np.ascontiguousarray(z.T)  # [NIN, B]
    WcT = np.ascontiguousarray(Wc.T)  # [NIN, G]
    W1T16 = np.ascontiguousarray(W1.T.astype(np.float16))  # [P, H]
    GS = G // NCORES
    wshT = np.ascontiguousarray(np.asarray(Wshape, f32).T)
    wdrT = np.ascontiguousarray(np.asarray(Wdrop, f32).T)
    bsh = np.asarray(bshape, f32).reshape(1, G)
    bdr = np.asarray(bdrop, f32).reshape(1, G)
    shared = {
        "zT": zT,
        "gamma": np.ascontiguousarray(gamma.reshape(H, 1)),
        "beta": np.ascontiguousarray(beta.reshape(H, 1)),
        "wscT": np.ascontiguousarray(np.asarray(Wscale, f32).T),
        "bsc": np.ascontiguousarray(np.asarray(bscale, f32).reshape(1, G)),
    }

    in_maps = []
    for r in range(NCORES):
        wcT_rot = np.zeros((NIN, G), f32)
        wcT_rot[:, : G - r] = WcT[:, r:]
        bc_rot = np.zeros((1, G), f32)
        bc_rot[0, : G - r] = bc[r:]
        w1t_r = np.zeros((PCPAD, H), np.float16)
        for (ll, lh, gl, gh) in maps[r]:
            w1t_r[ll:lh] = W1T16[gl:gh]
        # partition-major layout: [128, NPT*H], row p holds rows {t*128+p}
        w1t_r = np.ascontiguousarray(
            w1t_r.reshape(NPT, 128, H).transpose(1, 0, 2).reshape(128, NPT * H)
        )
        m = dict(shared)
        m["wcT_rot"] = wcT_rot
        m["bc_rot"] = bc_rot
        m["w1t"] = w1t_r
        m["wshT"] = np.ascontiguousarray(wshT[:, r * GS : (r + 1) * GS])
        m["wdrT"] = np.ascontiguousarray(wdrT[:, r * GS : (r + 1) * GS])
        m["bsh"] = np.ascontiguousarray(bsh[:, r * GS : (r + 1) * GS])
        m["bdr"] = np.ascontiguousarray(bdr[:, r * GS : (r + 1) * GS])
        in_maps.append(m)
    return in_maps


def _run(z, Wc, bc, W1, b1, gamma, beta, Wscale, bscale,
         Wshape, bshape, Wdrop, bdrop, trace=False, trace_cores=None):
    from concourse import bass_utils

    f32 = np.float32
    nc = _build_nc()
    maps = _host_maps()
    in_maps = _prep_in_maps(z, Wc, bc, W1, b1, gamma, beta, Wscale, bscale,
                            Wshape, bshape, Wdrop, bdrop)
    res = bass_utils.run_bass_kernel_spmd(
        nc, in_maps, core_ids=list(range(NCORES)),
        trace=trace, trace_cores=trace_cores,
    )
    outs = res.results

    px_sigma = np.empty((B, P), f32)
    for r in range(NCORES):
        sig_r = np.asarray(outs[r]["px_sig"], f32)
        for (ll, lh, gl, gh) in maps[r]:
            px_sigma[:, gl:gh] = sig_r[:, ll:lh]

    px_shape = np.concatenate([outs[r]["px_shape"] for r in range(NCORES)], axis=1)
    px_scale = outs[0]["px_scale"]
    px_dropout = np.concatenate([outs[r]["px_drop"] for r in range(NCORES)], axis=1)
    return (px_shape, px_scale, px_dropout, px_sigma), res


# revision 34
# speedup vs baseline: 766.5249x; 1.0071x over previous
"""Trainium2 Bass kernel for nn_DecoderVCSC (8-core SPMD).

Reference computation:
    c = z @ Wc.T + bc                                  (B, G)
    px_sigma = exp(-(c[:,iu] - c[:,jv]))               (B, P)   P = G*(G-1)/2
    h = px_sigma @ W1.T + b1                           (B, H)
    BN(training stats) + ReLU
    px_dropout = h @ Wdrop.T + bdrop
    px_shape   = h @ Wshape.T + bshape
    px_scale   = softmax(h @ Wscale.T + bscale)
    returns (px_shape, px_scale, px_dropout, px_sigma)

Sharding: the i<j pair blocks (block i = pairs (i, i+1..G-1)) are dealt
round-robin: core r handles blocks i = 8k + r, k = 0..127.  To keep the
SPMD instruction stream identical across cores, slot k is padded to
L_k = G-1-8k columns on every core (core r's true block length is
G-1-8k-r; the r pad columns are garbage on device and are (a) skipped by
the host unshard and (b) multiplied by host-zeroed W1T rows in the h
contraction).  The per-core shift r is absorbed on the host by passing a
rotated Wc (WcT_rot[:, t] = Wc[t+r, :]), so the device computes
cshift[b, t] = c[b, t+r] with core-independent addressing.

h is accumulated per-core over its p-slice and AllReduced across the 8
cores on-device; BN + heads + softmax run (redundantly) on every core.
b1 provably cancels in train-stats BN and is not sent to the device.
"""

import numpy as np

G = 1024
NIN = 64
H = 128
B = 256
P = G * (G - 1) // 2  # 523776
NCORES = 8
NSLOT = G // NCORES  # 128 slots per core
ALPHA = 1.0
BN_EPS = 1e-3

# Padded per-slot segment lengths/offsets (identical on all cores).
SEG_LEN = [G - 1 - NCORES * k for k in range(NSLOT)]  # 1023 - 8k
_off = np.cumsum([0] + SEG_LEN)
SEG_OFF = [int(x) for x in _off]
PCPAD = SEG_OFF[-1]  # 65920
NPT = PCPAD // 128  # 515 p-tiles of 128
assert PCPAD % 128 == 0

CHUNK = 2048  # sigma free-dim chunk width (elements)
CHUNKS = []
_c0 = 0
while _c0 < PCPAD:
    CHUNKS.append((_c0, min(_c0 + CHUNK, PCPAD)))
    _c0 += CHUNK


def _block_off(i):
    # global p offset of block i: sum_{t<i} (G-1-t)
    return i * (G - 1) - (i * (i - 1)) // 2


def _segments_in(lo, hi):
    """Yield (k, seg_lo, seg_hi) covering [lo,hi) split by slot boundaries."""
    out = []
    for k in range(NSLOT):
        s, e = SEG_OFF[k], SEG_OFF[k + 1]
        a, b = max(s, lo), min(e, hi)
        if a < b:
            out.append((k, a, b))
    return out


_NC_CACHE = {}


def _build_nc(single=False):
    key = "nc1" if single else "nc"
    if key in _NC_CACHE:
        return _NC_CACHE[key]
    import concourse.bass as bass
    import concourse.mybir as mybir
    import concourse.tile as tile
    from concourse import bacc
    from concourse.masks import make_identity

    f32 = mybir.dt.float32
    f32r = mybir.dt.float32r
    f16 = mybir.dt.float16
    AF = mybir.ActivationFunctionType
    ALU = mybir.AluOpType
    AX = mybir.AxisListType

    nc = bacc.Bacc(
        "TRN2", target_bir_lowering=False, debug=False,
        num_devices=1 if single else NCORES,
    )

    _pools = {}

    def _sgl(tc, shape, dtype, name, space="SBUF"):
        pool = _pools["psum" if space == "PSUM" else "const"]
        return pool.tile(shape, dtype, name=name, tag=name)

    # ---- I/O ----
    zT_d = nc.dram_tensor("zT", [NIN, B], f32, kind="ExternalInput").ap()
    wcT_d = nc.dram_tensor("wcT_rot", [NIN, G], f32, kind="ExternalInput").ap()
    bc_d = nc.dram_tensor("bc_rot", [1, G], f32, kind="ExternalInput").ap()
    w1t_d = nc.dram_tensor("w1t", [128, NPT * H], f16, kind="ExternalInput").ap()
    gamma_d = nc.dram_tensor("gamma", [H, 1], f32, kind="ExternalInput").ap()
    beta_d = nc.dram_tensor("beta", [H, 1], f32, kind="ExternalInput").ap()
    GS = G // NCORES  # per-core head-output columns
    wscT_d = nc.dram_tensor("wscT", [H, G], f32r, kind="ExternalInput").ap()
    wshT_d = nc.dram_tensor("wshT", [H, GS], f32r, kind="ExternalInput").ap()
    wdrT_d = nc.dram_tensor("wdrT", [H, GS], f32r, kind="ExternalInput").ap()
    bsc_d = nc.dram_tensor("bsc", [1, G], f32r, kind="ExternalInput").ap()
    bsh_d = nc.dram_tensor("bsh", [1, GS], f32r, kind="ExternalInput").ap()
    bdr_d = nc.dram_tensor("bdr", [1, GS], f32r, kind="ExternalInput").ap()

    psig_d = nc.dram_tensor("px_sig", [B, PCPAD], f16, kind="ExternalOutput").ap()
    psh_d = nc.dram_tensor("px_shape", [B, GS], f32, kind="ExternalOutput").ap()
    psc_d = nc.dram_tensor("px_scale", [B, G], f32, kind="ExternalOutput").ap()
    pdr_d = nc.dram_tensor("px_drop", [B, GS], f32, kind="ExternalOutput").ap()

    from contextlib import ExitStack

    with tile.TileContext(nc) as tc, ExitStack() as _stack:
        _pools["const"] = _stack.enter_context(tc.tile_pool(name="const", bufs=1))
        _pools["psum"] = _stack.enter_context(
            tc.tile_pool(name="constps", bufs=1, space="PSUM")
        )
        # ---------- constants ----------
        identF = _sgl(tc, [128, 128], f32, name="identF")
        make_identity(nc, identF[:])
        ident = _sgl(tc, [128, 128], f16, name="ident")
        nc.vector.tensor_copy(ident[:], identF[:])
        identr = ident[:]
        ones1cf = _sgl(tc, [128, 128], f32, name="ones1cf")
        nc.vector.memset(ones1cf[:], 1.0)
        ones1c = ones1cf[0:1, :]
        ones1f = _sgl(tc, [128, 128], f32r, name="ones1f")
        nc.vector.tensor_copy(ones1f[:], ones1cf[:])
        ones1 = ones1f[0:1, :]

        # ---------- small input loads ----------
        zT_sbf = _sgl(tc, [128, B], f32, name="zT_sbf")
        nc.sync.dma_start(zT_sbf[0:NIN, :], zT_d[:])
        zT_sb = zT_sbf[0:NIN, :]
        wcT_sbf = _sgl(tc, [128, G], f32, name="wcT_sbf")
        nc.sync.dma_start(wcT_sbf[0:NIN, :], wcT_d[:])
        wcT_sb = wcT_sbf[0:NIN, :]
        bc_sbf = _sgl(tc, [128, G], f32, name="bc_sbf")
        nc.sync.dma_start(bc_sbf[0:1, :], bc_d[:])
        bc_sb = bc_sbf[0:1, :]
        gamma_sb = _sgl(tc, [H, 1], f32, name="gamma_sb")
        nc.sync.dma_start(gamma_sb[:], gamma_d[:])
        beta_sb = _sgl(tc, [H, 1], f32, name="beta_sb")
        nc.sync.dma_start(beta_sb[:], beta_d[:])
        hw_sb = _sgl(tc, [H, G + 2 * GS], f32r, name="hw_sb")  # head weights (T)
        nc.sync.dma_start(hw_sb[:, 0:GS], wshT_d[:])
        nc.sync.dma_start(hw_sb[:, GS : 2 * GS], wdrT_d[:])
        nc.sync.dma_start(hw_sb[:, 2 * GS : 2 * GS + G], wscT_d[:])
        hb_sbf = _sgl(tc, [128, G + 2 * GS], f32r, name="hb_sbf")  # head biases
        hb_sb = hb_sbf[0:1, :]
        nc.sync.dma_start(hb_sbf[0:1, 0:GS], bsh_d[:])
        nc.sync.dma_start(hb_sbf[0:1, GS : 2 * GS], bdr_d[:])
        nc.sync.dma_start(hb_sbf[0:1, 2 * GS : 2 * GS + G], bsc_d[:])

        # ---------- cshift = (z @ WcT_rot) + bc_rot,  [2x128 b, 1024 g'] ----------
        ebuf = []  # exp(cshift)
        fbuf = []  # exp(-cshift)
        with tc.tile_pool(name="cpsum", bufs=4, space="PSUM") as cps_pool:
            for bt in range(2):
                e_sb = _sgl(tc, [128, G], f32, name=f"ebuf{bt}")
                f_sb = _sgl(tc, [128, G], f32, name=f"fbuf{bt}")
                for gh in range(2):  # two 512-wide psum tiles
                    cps = cps_pool.tile([128, 512], f32, tag="cps", name="cps")
                    gsl = slice(gh * 512, (gh + 1) * 512)
                    # bias broadcast via rank-1 matmul, then accumulate z@WcT
                    nc.tensor.matmul(
                        cps[:], ones1c[:, 0:128], bc_sb[:, gsl], start=True, stop=False
                    )
                    nc.tensor.matmul(
                        cps[:],
                        zT_sb[:, bt * 128 : (bt + 1) * 128],
                        wcT_sb[:, gsl],
                        start=False,
                        stop=True,
                    )
                    nc.scalar.activation(e_sb[:, gsl], cps[:], AF.Exp)
                    nc.scalar.activation(f_sb[:, gsl], cps[:], AF.Exp, scale=-1.0)
                ebuf.append(e_sb)
                fbuf.append(f_sb)

        # ---------- main loop: sigma gen -> HBM + transpose -> h matmul ----------
        h_ps = _sgl(tc, [128, B], f32, name="h_ps", space="PSUM")  # h.T accumulator
        pt_global = 0
        with (
            tc.tile_pool(name="sig", bufs=3) as sig_pool,
            tc.tile_pool(name="sigT", bufs=4) as sigT_pool,
            tc.tile_pool(name="sigTps", bufs=4, space="PSUM") as sigTps_pool,
            tc.tile_pool(name="w1", bufs=12) as w1_pool,
        ):
            for (c0, c1) in CHUNKS:
                W = c1 - c0
                nt = W // 128
                # -- sigma generation (ACT), both b-tiles --
                sig = [
                    sig_pool.tile([128, W], f16, tag=f"sig{bt}", name=f"sig{bt}") for bt in range(2)
                ]
                for (k, a, b) in _segments_in(c0, c1):
                    # sigma[:, a:b] = exp(cshift[:, 8k+1+(a-S_k) : ...] - cshift[:, 8k])
                    j0 = NCORES * k + 1 + (a - SEG_OFF[k])
                    for bt in range(2):
                        nc.vector.tensor_scalar_mul(
                            sig[bt][:, a - c0 : b - c0],
                            ebuf[bt][:, j0 : j0 + (b - a)],
                            fbuf[bt][:, NCORES * k : NCORES * k + 1],
                        )
                # -- write sigma chunk to HBM --
                for bt in range(2):
                    nc.sync.dma_start(
                        psig_d[bt * 128 : (bt + 1) * 128, c0:c1], sig[bt][:]
                    )
                # -- W1T chunk load: [W, H] rows -> SBUF [128, nt*H] --
                w1c = w1_pool.tile([128, nt * H], f16, tag="w1c", name="w1c")
                nc.gpsimd.dma_start(
                    w1c[:], w1t_d[:, (c0 // 128) * H : (c1 // 128) * H]
                )
                # -- per p-tile: transpose sigma, accumulate h --
                for t2 in range(0, nt, 2):
                    npair = min(2, nt - t2)
                    tps = sigTps_pool.tile([128, 512], f16, tag="tps", name="tps")
                    for ti in range(npair):
                        for bt in range(2):
                            nc.tensor.transpose(
                                tps[:, ti * 256 + bt * 128 : ti * 256 + (bt + 1) * 128],
                                sig[bt][:, (t2 + ti) * 128 : (t2 + ti + 1) * 128],
                                identr[:],
                            )
                    sT = sigT_pool.tile([128, 512], f16, tag="sT", name="sT")
                    if (t2 // 2) % 5 == 0:
                        nc.vector.tensor_copy(
                            sT[:, : npair * 256], tps[:, : npair * 256]
                        )
                    else:
                        nc.scalar.copy(
                            sT[:, : npair * 256], tps[:, : npair * 256]
                        )
                    for ti in range(npair):
                        t = t2 + ti
                        nc.tensor.matmul(
                            h_ps[:],
                            w1c[:, t * H : (t + 1) * H],
                            sT[:, ti * 256 : (ti + 1) * 256],
                            start=(pt_global == 0),
                            stop=(pt_global == NPT - 1),
                            skip_group_check=True,
                        )
                        pt_global += 1

        # ---------- AllReduce h across cores ----------
        h_sb = _sgl(tc, [128, B], f32, name="h_sb")
        nc.scalar.copy(h_sb[:], h_ps[:])
        with tc.tile_pool(name="dram", bufs=2, space="DRAM") as dram:
            h_in = dram.tile([128, B], f32, name="hbounce")
            h_out = dram.tile([128, B], f32, name="hbounce")
            nc.sync.dma_start(h_in[:], h_sb[:])
            if single:
                # cost-model variant: stand in for the AllReduce with a copy
                nc.gpsimd.dma_start(h_out[:], h_in[:])
            else:
                nc.gpsimd.collective_compute(
                    "AllReduce",
                    ALU.add,
                    replica_groups=[list(range(NCORES))],
                    ins=[h_in[:].opt()],
                    outs=[h_out[:].opt()],
                )
            hall = _sgl(tc, [128, B], f32, name="hall")
            nc.sync.dma_start(hall[:], h_out[:])

        # ---------- BatchNorm (training stats) + ReLU, in [k, b] layout ----------
        musum = _sgl(tc, [128, 1], f32, name="musum")
        nc.vector.tensor_reduce(musum[:], hall[:], axis=AX.X, op=ALU.add)
        hsq = _sgl(tc, [128, B], f32, name="hsq")
        nc.vector.tensor_tensor(hsq[:], hall[:], hall[:], op=ALU.mult)
        sqsum = _sgl(tc, [128, 1], f32, name="sqsum")
        nc.vector.tensor_reduce(sqsum[:], hsq[:], axis=AX.X, op=ALU.add)
        mu = _sgl(tc, [128, 1], f32, name="mu")
        nc.vector.tensor_scalar_mul(mu[:], musum[:], 1.0 / B)
        musq = _sgl(tc, [128, 1], f32, name="musq")
        nc.vector.tensor_tensor(musq[:], mu[:], mu[:], op=ALU.mult)
        var = _sgl(tc, [128, 1], f32, name="var")
        nc.vector.scalar_tensor_tensor(
            var[:], sqsum[:], 1.0 / B, musq[:],
            op0=ALU.mult, op1=ALU.subtract,
        )
        varp = _sgl(tc, [128, 1], f32, name="varp")
        nc.vector.tensor_scalar_add(varp[:], var[:], BN_EPS)
        i32 = mybir.dt.int32
        magic = _sgl(tc, [128, 1], i32, name="magic")
        nc.vector.memset(magic[:], 0x5F3759DF)
        ihalf = _sgl(tc, [128, 1], i32, name="ihalf")
        nc.vector.tensor_scalar(
            ihalf[:], varp[:].bitcast(i32), 1, None, op0=ALU.arith_shift_right
        )
        yint = _sgl(tc, [128, 1], i32, name="yint")
        nc.vector.tensor_tensor(yint[:], magic[:], ihalf[:], op=ALU.subtract)
        rst = yint[:].bitcast(f32)
        for _it in range(3):
            nt1 = _sgl(tc, [128, 1], f32, name=f"nt1_{_it}")
            nc.vector.tensor_tensor(nt1[:], varp[:], rst, op=ALU.mult)
            nt2 = _sgl(tc, [128, 1], f32, name=f"nt2_{_it}")
            nc.vector.tensor_tensor(nt2[:], nt1[:], rst, op=ALU.mult)
            nt3 = _sgl(tc, [128, 1], f32, name=f"nt3_{_it}")
            nc.vector.tensor_scalar(
                nt3[:], nt2[:], -0.5, 1.5, op0=ALU.mult, op1=ALU.add
            )
            nt4 = _sgl(tc, [128, 1], f32, name=f"nt4_{_it}")
            nc.vector.tensor_tensor(nt4[:], nt3[:], rst, op=ALU.mult)
            rst = nt4[:]
        bnsc = _sgl(tc, [128, 1], f32, name="bnsc")
        nc.vector.tensor_tensor(bnsc[:], gamma_sb[:], rst, op=ALU.mult)
        mbs = _sgl(tc, [128, 1], f32, name="mbs")
        nc.vector.tensor_tensor(mbs[:], mu[:], bnsc[:], op=ALU.mult)
        bnbi = _sgl(tc, [128, 1], f32, name="bnbi")
        nc.vector.tensor_tensor(bnbi[:], beta_sb[:], mbs[:], op=ALU.subtract)
        hbn = _sgl(tc, [128, B], f32r, name="hbn")
        nc.scalar.activation(hbn[:], hall[:], AF.Relu, bias=bnbi[:], scale=bnsc[:])

        # ---------- heads ----------
        # sharded shape/drop: this core computes its GS output columns
        with (
            tc.tile_pool(name="hps", bufs=3, space="PSUM") as hps_pool,
            tc.tile_pool(name="hout", bufs=4) as hout_pool,
        ):
            for hd, (out_d, ev_eng) in enumerate(
                [(psh_d, nc.scalar), (pdr_d, nc.vector)]
            ):
                off = hd * GS
                for bt in range(2):
                    hps = hps_pool.tile([128, 512], f32, tag="hps", name="hps")[:, 0:GS]
                    nc.tensor.matmul(
                        hps[:], ones1[:, 0:128], hb_sb[:, off : off + GS],
                        start=True, stop=False,
                    )
                    nc.tensor.matmul(
                        hps[:],
                        hbn[:, bt * 128 : (bt + 1) * 128],
                        hw_sb[:, off : off + GS],
                        start=False,
                        stop=True,
                    )
                    o_sb = hout_pool.tile([128, GS], f32, tag="osm", name="o_sb")
                    if hd == 0:
                        ev_eng.copy(o_sb[:], hps[:])
                    else:
                        ev_eng.tensor_copy(o_sb[:], hps[:])
                    nc.sync.dma_start(out_d[bt * 128 : (bt + 1) * 128, :], o_sb[:])
            # scale head: full G + softmax, replicated on every core
            for bt in range(2):
                o_sb = hout_pool.tile([128, G], f32, tag="o_sb", name="o_sb")
                for gh in range(2):
                    hps = hps_pool.tile([128, 512], f32, tag="hps", name="hps")
                    gsl = slice(2 * GS + gh * 512, 2 * GS + (gh + 1) * 512)
                    nc.tensor.matmul(
                        hps[:], ones1[:, 0:128], hb_sb[:, gsl],
                        start=True, stop=False,
                    )
                    nc.tensor.matmul(
                        hps[:],
                        hbn[:, bt * 128 : (bt + 1) * 128],
                        hw_sb[:, gsl],
                        start=False,
                        stop=True,
                    )
                    if gh == 0:
                        nc.scalar.copy(o_sb[:, gh * 512 : (gh + 1) * 512], hps[:])
                    else:
                        nc.vector.tensor_copy(
                            o_sb[:, gh * 512 : (gh + 1) * 512], hps[:]
                        )
                mx = hout_pool.tile([128, 1], f32, tag="mx", name="mx")
                nc.vector.tensor_reduce(mx[:], o_sb[:], axis=AX.X, op=ALU.max)
                nmx = hout_pool.tile([128, 1], f32, tag="nmx", name="nmx")
                nc.vector.tensor_scalar_mul(nmx[:], mx[:], -1.0)
                exl = hout_pool.tile([128, G], f32, tag="exl", name="exl")
                nc.scalar.activation(exl[:], o_sb[:], AF.Exp, bias=nmx[:])
                ssum = hout_pool.tile([128, 1], f32, tag="ssum", name="ssum")
                nc.vector.tensor_reduce(ssum[:], exl[:], axis=AX.X, op=ALU.add)
                sinv = hout_pool.tile([128, 1], f32, tag="sinv", name="sinv")
                nc.vector.reciprocal(sinv[:], ssum[:])
                smx = hout_pool.tile([128, G], f32, tag="smx", name="smx")
                nc.vector.tensor_scalar_mul(smx[:], exl[:], sinv[:])
                nc.sync.dma_start(psc_d[bt * 128 : (bt + 1) * 128, :], smx[:])

    nc.compile()
    _NC_CACHE[key] = nc
    return nc


_HOST_CACHE = {}


def _host_maps():
    """Static per-core index maps (depend only on shapes)."""
    if "maps" in _HOST_CACHE:
        return _HOST_CACHE["maps"]
    # per core r, per slot k: true i = 8k+r, valid len = G-1-i, local seg at SEG_OFF[k]
    maps = []
    for r in range(NCORES):
        rows = []  # (local_lo, local_hi, global_lo, global_hi)
        for k in range(NSLOT):
            i = NCORES * k + r
            vlen = G - 1 - i
            if vlen <= 0:
                continue
            o = _block_off(i)
            rows.append((SEG_OFF[k], SEG_OFF[k] + vlen, o, o + vlen))
        maps.append(rows)
    _HOST_CACHE["maps"] = maps
    return maps


def kernel(z, Wc, bc, W1, b1, gamma, beta, Wscale, bscale,
           Wshape, bshape, Wdrop, bdrop):
    return _run(z, Wc, bc, W1, b1, gamma, beta, Wscale, bscale,
                Wshape, bshape, Wdrop, bdrop)[0]


def _prep_in_maps(z, Wc, bc, W1, b1, gamma, beta, Wscale, bscale,
                  Wshape, bshape, Wdrop, bdrop):
    f32 = np.float32
    z = np.asarray(z, f32)
    Wc = np.asarray(Wc, f32)
    bc = np.asarray(bc, f32)
    W1 = np.asarray(W1, f32)
    gamma = np.asarray(gamma, f32)
    beta = np.asarray(beta, f32)
    maps = _host_maps()

    zT = np.ascontiguousarray(z.T)  # [NIN, B]
    WcT = np.ascontiguousarray(Wc.T)  # [NIN, G]
    W1T16 = np.ascontiguousarray(W1.T.astype(np.float16))  # [P, H]
    GS = G // NCORES
    wshT = np.ascontiguousarray(np.asarray(Wshape, f32).T)
    wdrT = np.ascontiguousarray(np.asarray(Wdrop, f32).T)
    bsh = np.asarray(bshape, f32).reshape(1, G)
    bdr = np.asarray(bdrop, f32).reshape(1, G)
    shared = {
        "zT": zT,
        "gamma": np.ascontiguousarray(gamma.reshape(H, 1)),
        "beta": np.ascontiguousarray(beta.reshape(H, 1)),
        "wscT": np.ascontiguousarray(np.asarray(Wscale, f32).T),
        "bsc": np.ascontiguousarray(np.asarray(bscale, f32).reshape(1, G)),
    }

    in_maps = []
    for r in range(NCORES):
        wcT_rot = np.zeros((NIN, G), f32)
        wcT_rot[:, : G - r] = WcT[:, r:]
        bc_rot = np.zeros((1, G), f32)
        bc_rot[0, : G - r] = bc[r:]
        w1t_r = np.zeros((PCPAD, H), np.float16)
        for (ll, lh, gl, gh) in maps[r]:
            w1t_r[ll:lh] = W1T16[gl:gh]
        # partition-major layout: [128, NPT*H], row p holds rows {t*128+p}
        w1t_r = np.ascontiguousarray(
            w1t_r.reshape(NPT, 128, H).transpose(1, 0, 2).reshape(128, NPT * H)
        )
        m = dict(shared)
        m["wcT_rot"] = wcT_rot
        m["bc_rot"] = bc_rot
        m["w1t"] = w1t_r
        m["wshT"] = np.ascontiguousarray(wshT[:, r * GS : (r + 1) * GS])
        m["wdrT"] = np.ascontiguousarray(wdrT[:, r * GS : (r + 1) * GS])
        m["bsh"] = np.ascontiguousarray(bsh[:, r * GS : (r + 1) * GS])
        m["bdr"] = np.ascontiguousarray(bdr[:, r * GS : (r + 1) * GS])
        in_maps.append(m)
    return in_maps


def _run(z, Wc, bc, W1, b1, gamma, beta, Wscale, bscale,
         Wshape, bshape, Wdrop, bdrop, trace=False, trace_cores=None):
    from concourse import bass_utils

    f32 = np.float32
    nc = _build_nc()
    maps = _host_maps()
    in_maps = _prep_in_maps(z, Wc, bc, W1, b1, gamma, beta, Wscale, bscale,
                            Wshape, bshape, Wdrop, bdrop)
    res = bass_utils.run_bass_kernel_spmd(
        nc, in_maps, core_ids=list(range(NCORES)),
        trace=trace, trace_cores=trace_cores,
    )
    outs = res.results

    px_sigma = np.empty((B, P), f32)
    for r in range(NCORES):
        sig_r = np.asarray(outs[r]["px_sig"], f32)
        for (ll, lh, gl, gh) in maps[r]:
            px_sigma[:, gl:gh] = sig_r[:, ll:lh]

    px_shape = np.concatenate([outs[r]["px_shape"] for r in range(NCORES)], axis=1)
    px_scale = outs[0]["px_scale"]
    px_dropout = np.concatenate([outs[r]["px_drop"] for r in range(NCORES)], axis=1)
    return (px_shape, px_scale, px_dropout, px_sigma), res


# revision 38
# speedup vs baseline: 772.6995x; 1.0081x over previous
"""Trainium2 Bass kernel for nn_DecoderVCSC (8-core SPMD).

Reference computation:
    c = z @ Wc.T + bc                                  (B, G)
    px_sigma = exp(-(c[:,iu] - c[:,jv]))               (B, P)   P = G*(G-1)/2
    h = px_sigma @ W1.T + b1                           (B, H)
    BN(training stats) + ReLU
    px_dropout = h @ Wdrop.T + bdrop
    px_shape   = h @ Wshape.T + bshape
    px_scale   = softmax(h @ Wscale.T + bscale)
    returns (px_shape, px_scale, px_dropout, px_sigma)

Sharding: the i<j pair blocks (block i = pairs (i, i+1..G-1)) are dealt
round-robin: core r handles blocks i = 8k + r, k = 0..127.  To keep the
SPMD instruction stream identical across cores, slot k is padded to
L_k = G-1-8k columns on every core (core r's true block length is
G-1-8k-r; the r pad columns are garbage on device and are (a) skipped by
the host unshard and (b) multiplied by host-zeroed W1T rows in the h
contraction).  The per-core shift r is absorbed on the host by passing a
rotated Wc (WcT_rot[:, t] = Wc[t+r, :]), so the device computes
cshift[b, t] = c[b, t+r] with core-independent addressing.

h is accumulated per-core over its p-slice and AllReduced across the 8
cores on-device; BN + heads + softmax run (redundantly) on every core.
b1 provably cancels in train-stats BN and is not sent to the device.
"""

import numpy as np

G = 1024
NIN = 64
H = 128
B = 256
P = G * (G - 1) // 2  # 523776
NCORES = 8
NSLOT = G // NCORES  # 128 slots per core
ALPHA = 1.0
BN_EPS = 1e-3

# Padded per-slot segment lengths/offsets (identical on all cores).
SEG_LEN = [G - 1 - NCORES * k for k in range(NSLOT)]  # 1023 - 8k
_off = np.cumsum([0] + SEG_LEN)
SEG_OFF = [int(x) for x in _off]
PCPAD = SEG_OFF[-1]  # 65920
NPT = PCPAD // 128  # 515 p-tiles of 128
assert PCPAD % 128 == 0

CHUNK = 2048  # sigma free-dim chunk width (elements)
CHUNKS = []
_c0 = 0
while _c0 < PCPAD:
    CHUNKS.append((_c0, min(_c0 + CHUNK, PCPAD)))
    _c0 += CHUNK


def _block_off(i):
    # global p offset of block i: sum_{t<i} (G-1-t)
    return i * (G - 1) - (i * (i - 1)) // 2


def _segments_in(lo, hi):
    """Yield (k, seg_lo, seg_hi) covering [lo,hi) split by slot boundaries."""
    out = []
    for k in range(NSLOT):
        s, e = SEG_OFF[k], SEG_OFF[k + 1]
        a, b = max(s, lo), min(e, hi)
        if a < b:
            out.append((k, a, b))
    return out


_NC_CACHE = {}


def _build_nc(single=False):
    key = "nc1" if single else "nc"
    if key in _NC_CACHE:
        return _NC_CACHE[key]
    import concourse.bass as bass
    import concourse.mybir as mybir
    import concourse.tile as tile
    from concourse import bacc
    from concourse.masks import make_identity

    f32 = mybir.dt.float32
    f32r = mybir.dt.float32r
    f16 = mybir.dt.float16
    AF = mybir.ActivationFunctionType
    ALU = mybir.AluOpType
    AX = mybir.AxisListType

    nc = bacc.Bacc(
        "TRN2", target_bir_lowering=False, debug=False,
        num_devices=1 if single else NCORES,
    )

    _pools = {}

    def _sgl(tc, shape, dtype, name, space="SBUF"):
        pool = _pools["psum" if space == "PSUM" else "const"]
        return pool.tile(shape, dtype, name=name, tag=name)

    # ---- I/O ----
    zT_d = nc.dram_tensor("zT", [NIN, B], f32, kind="ExternalInput").ap()
    wcT_d = nc.dram_tensor("wcT_rot", [NIN, G], f32, kind="ExternalInput").ap()
    bc_d = nc.dram_tensor("bc_rot", [1, G], f32, kind="ExternalInput").ap()
    w1t_d = nc.dram_tensor("w1t", [128, NPT * H], f16, kind="ExternalInput").ap()
    gamma_d = nc.dram_tensor("gamma", [H, 1], f32, kind="ExternalInput").ap()
    beta_d = nc.dram_tensor("beta", [H, 1], f32, kind="ExternalInput").ap()
    GS = G // NCORES  # per-core head-output columns
    wscT_d = nc.dram_tensor("wscT", [H, G], f32r, kind="ExternalInput").ap()
    wshT_d = nc.dram_tensor("wshT", [H, GS], f32r, kind="ExternalInput").ap()
    wdrT_d = nc.dram_tensor("wdrT", [H, GS], f32r, kind="ExternalInput").ap()
    bsc_d = nc.dram_tensor("bsc", [1, G], f32r, kind="ExternalInput").ap()
    bsh_d = nc.dram_tensor("bsh", [1, GS], f32r, kind="ExternalInput").ap()
    bdr_d = nc.dram_tensor("bdr", [1, GS], f32r, kind="ExternalInput").ap()

    psig_d = nc.dram_tensor("px_sig", [B, PCPAD], f16, kind="ExternalOutput").ap()
    psh_d = nc.dram_tensor("px_shape", [B, GS], f32, kind="ExternalOutput").ap()
    psc_d = nc.dram_tensor("px_scale", [B, G], f32, kind="ExternalOutput").ap()
    pdr_d = nc.dram_tensor("px_drop", [B, GS], f32, kind="ExternalOutput").ap()

    from contextlib import ExitStack

    with tile.TileContext(nc) as tc, ExitStack() as _stack:
        _pools["const"] = _stack.enter_context(tc.tile_pool(name="const", bufs=1))
        _pools["psum"] = _stack.enter_context(
            tc.tile_pool(name="constps", bufs=1, space="PSUM")
        )
        # ---------- constants ----------
        identF = _sgl(tc, [128, 128], f32, name="identF")
        make_identity(nc, identF[:])
        ident = _sgl(tc, [128, 128], f16, name="ident")
        nc.vector.tensor_copy(ident[:], identF[:])
        identr = ident[:]
        ones1cf = _sgl(tc, [128, 128], f32, name="ones1cf")
        nc.vector.memset(ones1cf[:], 1.0)
        ones1c = ones1cf[0:1, :]
        ones1f = _sgl(tc, [128, 128], f32r, name="ones1f")
        nc.vector.tensor_copy(ones1f[:], ones1cf[:])
        ones1 = ones1f[0:1, :]

        # ---------- small input loads ----------
        zT_sbf = _sgl(tc, [128, B], f32, name="zT_sbf")
        nc.sync.dma_start(zT_sbf[0:NIN, :], zT_d[:])
        zT_sb = zT_sbf[0:NIN, :]
        wcT_sbf = _sgl(tc, [128, G], f32, name="wcT_sbf")
        nc.sync.dma_start(wcT_sbf[0:NIN, :], wcT_d[:])
        wcT_sb = wcT_sbf[0:NIN, :]
        bc_sbf = _sgl(tc, [128, G], f32, name="bc_sbf")
        nc.sync.dma_start(bc_sbf[0:1, :], bc_d[:])
        bc_sb = bc_sbf[0:1, :]
        gamma_sb = _sgl(tc, [H, 1], f32, name="gamma_sb")
        nc.sync.dma_start(gamma_sb[:], gamma_d[:])
        beta_sb = _sgl(tc, [H, 1], f32, name="beta_sb")
        nc.sync.dma_start(beta_sb[:], beta_d[:])
        hw_sb = _sgl(tc, [H, G + 2 * GS], f32r, name="hw_sb")  # head weights (T)
        nc.sync.dma_start(hw_sb[:, 0:GS], wshT_d[:])
        nc.sync.dma_start(hw_sb[:, GS : 2 * GS], wdrT_d[:])
        nc.sync.dma_start(hw_sb[:, 2 * GS : 2 * GS + G], wscT_d[:])
        hb_sbf = _sgl(tc, [128, G + 2 * GS], f32r, name="hb_sbf")  # head biases
        hb_sb = hb_sbf[0:1, :]
        nc.sync.dma_start(hb_sbf[0:1, 0:GS], bsh_d[:])
        nc.sync.dma_start(hb_sbf[0:1, GS : 2 * GS], bdr_d[:])
        nc.sync.dma_start(hb_sbf[0:1, 2 * GS : 2 * GS + G], bsc_d[:])

        # ---------- cshift = (z @ WcT_rot) + bc_rot,  [2x128 b, 1024 g'] ----------
        ebuf = []  # exp(cshift)
        fbuf = []  # exp(-cshift)
        with tc.tile_pool(name="cpsum", bufs=4, space="PSUM") as cps_pool:
            for bt in range(2):
                e_sb = _sgl(tc, [128, G], f32, name=f"ebuf{bt}")
                f_sb = _sgl(tc, [128, G], f32, name=f"fbuf{bt}")
                for gh in range(2):  # two 512-wide psum tiles
                    cps = cps_pool.tile([128, 512], f32, tag="cps", name="cps")
                    gsl = slice(gh * 512, (gh + 1) * 512)
                    # bias broadcast via rank-1 matmul, then accumulate z@WcT
                    nc.tensor.matmul(
                        cps[:], ones1c[:, 0:128], bc_sb[:, gsl], start=True, stop=False
                    )
                    nc.tensor.matmul(
                        cps[:],
                        zT_sb[:, bt * 128 : (bt + 1) * 128],
                        wcT_sb[:, gsl],
                        start=False,
                        stop=True,
                    )
                    nc.scalar.activation(e_sb[:, gsl], cps[:], AF.Exp)
                    nc.scalar.activation(f_sb[:, gsl], cps[:], AF.Exp, scale=-1.0)
                ebuf.append(e_sb)
                fbuf.append(f_sb)

        # ---------- main loop: sigma gen -> HBM + transpose -> h matmul ----------
        h_ps = _sgl(tc, [128, B], f32, name="h_ps", space="PSUM")  # h.T accumulator
        pt_global = 0
        with (
            tc.tile_pool(name="sig", bufs=4) as sig_pool,
            tc.tile_pool(name="sigT", bufs=6) as sigT_pool,
            tc.tile_pool(name="sigTps", bufs=6, space="PSUM") as sigTps_pool,
            tc.tile_pool(name="w1", bufs=12) as w1_pool,
        ):
            for (c0, c1) in CHUNKS:
                W = c1 - c0
                nt = W // 128
                # -- sigma generation (ACT), both b-tiles --
                sig = [
                    sig_pool.tile([128, W], f16, tag=f"sig{bt}", name=f"sig{bt}") for bt in range(2)
                ]
                for (k, a, b) in _segments_in(c0, c1):
                    # sigma[:, a:b] = exp(cshift[:, 8k+1+(a-S_k) : ...] - cshift[:, 8k])
                    j0 = NCORES * k + 1 + (a - SEG_OFF[k])
                    for bt in range(2):
                        nc.vector.tensor_scalar_mul(
                            sig[bt][:, a - c0 : b - c0],
                            ebuf[bt][:, j0 : j0 + (b - a)],
                            fbuf[bt][:, NCORES * k : NCORES * k + 1],
                        )
                # -- write sigma chunk to HBM --
                for bt in range(2):
                    nc.sync.dma_start(
                        psig_d[bt * 128 : (bt + 1) * 128, c0:c1], sig[bt][:]
                    )
                # -- W1T chunk load: [W, H] rows -> SBUF [128, nt*H] --
                w1c = w1_pool.tile([128, nt * H], f16, tag="w1c", name="w1c")
                nc.gpsimd.dma_start(
                    w1c[:], w1t_d[:, (c0 // 128) * H : (c1 // 128) * H]
                )
                # -- per p-tile: transpose sigma, accumulate h --
                for t2 in range(0, nt, 2):
                    npair = min(2, nt - t2)
                    tps = sigTps_pool.tile([128, 512], f16, tag="tps", name="tps")
                    for ti in range(npair):
                        for bt in range(2):
                            nc.tensor.transpose(
                                tps[:, ti * 256 + bt * 128 : ti * 256 + (bt + 1) * 128],
                                sig[bt][:, (t2 + ti) * 128 : (t2 + ti + 1) * 128],
                                identr[:],
                            )
                    sT = sigT_pool.tile([128, 512], f16, tag="sT", name="sT")
                    if (t2 // 2) % 5 == 0:
                        nc.vector.tensor_copy(
                            sT[:, : npair * 256], tps[:, : npair * 256]
                        )
                    else:
                        nc.scalar.copy(
                            sT[:, : npair * 256], tps[:, : npair * 256]
                        )
                    for ti in range(npair):
                        t = t2 + ti
                        nc.tensor.matmul(
                            h_ps[:],
                            w1c[:, t * H : (t + 1) * H],
                            sT[:, ti * 256 : (ti + 1) * 256],
                            start=(pt_global == 0),
                            stop=(pt_global == NPT - 1),
                            skip_group_check=True,
                        )
                        pt_global += 1

        # ---------- AllReduce h across cores ----------
        h_sb = _sgl(tc, [128, B], f32, name="h_sb")
        nc.scalar.copy(h_sb[:], h_ps[:])
        with tc.tile_pool(name="dram", bufs=2, space="DRAM") as dram:
            h_in = dram.tile([128, B], f32, name="hbounce")
            h_out = dram.tile([128, B], f32, name="hbounce")
            nc.sync.dma_start(h_in[:], h_sb[:])
            if single:
                # cost-model variant: stand in for the AllReduce with a copy
                nc.gpsimd.dma_start(h_out[:], h_in[:])
            else:
                nc.gpsimd.collective_compute(
                    "AllReduce",
                    ALU.add,
                    replica_groups=[list(range(NCORES))],
                    ins=[h_in[:].opt()],
                    outs=[h_out[:].opt()],
                )
            hall = _sgl(tc, [128, B], f32, name="hall")
            nc.sync.dma_start(hall[:], h_out[:])

        # ---------- BatchNorm (training stats) + ReLU, in [k, b] layout ----------
        musum = _sgl(tc, [128, 1], f32, name="musum")
        nc.vector.tensor_reduce(musum[:], hall[:], axis=AX.X, op=ALU.add)
        hsq = _sgl(tc, [128, B], f32, name="hsq")
        nc.vector.tensor_tensor(hsq[:], hall[:], hall[:], op=ALU.mult)
        sqsum = _sgl(tc, [128, 1], f32, name="sqsum")
        nc.vector.tensor_reduce(sqsum[:], hsq[:], axis=AX.X, op=ALU.add)
        mu = _sgl(tc, [128, 1], f32, name="mu")
        nc.vector.tensor_scalar_mul(mu[:], musum[:], 1.0 / B)
        musq = _sgl(tc, [128, 1], f32, name="musq")
        nc.vector.tensor_tensor(musq[:], mu[:], mu[:], op=ALU.mult)
        var = _sgl(tc, [128, 1], f32, name="var")
        nc.vector.scalar_tensor_tensor(
            var[:], sqsum[:], 1.0 / B, musq[:],
            op0=ALU.mult, op1=ALU.subtract,
        )
        varp = _sgl(tc, [128, 1], f32, name="varp")
        nc.vector.tensor_scalar_add(varp[:], var[:], BN_EPS)
        i32 = mybir.dt.int32
        magic = _sgl(tc, [128, 1], i32, name="magic")
        nc.vector.memset(magic[:], 0x5F3759DF)
        ihalf = _sgl(tc, [128, 1], i32, name="ihalf")
        nc.vector.tensor_scalar(
            ihalf[:], varp[:].bitcast(i32), 1, None, op0=ALU.arith_shift_right
        )
        yint = _sgl(tc, [128, 1], i32, name="yint")
        nc.vector.tensor_tensor(yint[:], magic[:], ihalf[:], op=ALU.subtract)
        rst = yint[:].bitcast(f32)
        for _it in range(3):
            nt1 = _sgl(tc, [128, 1], f32, name=f"nt1_{_it}")
            nc.vector.tensor_tensor(nt1[:], varp[:], rst, op=ALU.mult)
            nt2 = _sgl(tc, [128, 1], f32, name=f"nt2_{_it}")
            nc.vector.tensor_tensor(nt2[:], nt1[:], rst, op=ALU.mult)
            nt3 = _sgl(tc, [128, 1], f32, name=f"nt3_{_it}")
            nc.vector.tensor_scalar(
                nt3[:], nt2[:], -0.5, 1.5, op0=ALU.mult, op1=ALU.add
            )
            nt4 = _sgl(tc, [128, 1], f32, name=f"nt4_{_it}")
            nc.vector.tensor_tensor(nt4[:], nt3[:], rst, op=ALU.mult)
            rst = nt4[:]
        bnsc = _sgl(tc, [128, 1], f32, name="bnsc")
        nc.vector.tensor_tensor(bnsc[:], gamma_sb[:], rst, op=ALU.mult)
        mbs = _sgl(tc, [128, 1], f32, name="mbs")
        nc.vector.tensor_tensor(mbs[:], mu[:], bnsc[:], op=ALU.mult)
        bnbi = _sgl(tc, [128, 1], f32, name="bnbi")
        nc.vector.tensor_tensor(bnbi[:], beta_sb[:], mbs[:], op=ALU.subtract)
        hbn = _sgl(tc, [128, B], f32r, name="hbn")
        nc.scalar.activation(hbn[:], hall[:], AF.Relu, bias=bnbi[:], scale=bnsc[:])

        # ---------- heads ----------
        # sharded shape/drop: this core computes its GS output columns
        with (
            tc.tile_pool(name="hps", bufs=3, space="PSUM") as hps_pool,
            tc.tile_pool(name="hout", bufs=4) as hout_pool,
        ):
            for hd, (out_d, ev_eng) in enumerate(
                [(psh_d, nc.scalar), (pdr_d, nc.vector)]
            ):
                off = hd * GS
                for bt in range(2):
                    hps = hps_pool.tile([128, 512], f32, tag="hps", name="hps")[:, 0:GS]
                    nc.tensor.matmul(
                        hps[:], ones1[:, 0:128], hb_sb[:, off : off + GS],
                        start=True, stop=False,
                    )
                    nc.tensor.matmul(
                        hps[:],
                        hbn[:, bt * 128 : (bt + 1) * 128],
                        hw_sb[:, off : off + GS],
                        start=False,
                        stop=True,
                    )
                    o_sb = hout_pool.tile([128, GS], f32, tag="osm", name="o_sb")
                    if hd == 0:
                        ev_eng.copy(o_sb[:], hps[:])
                    else:
                        ev_eng.tensor_copy(o_sb[:], hps[:])
                    nc.sync.dma_start(out_d[bt * 128 : (bt + 1) * 128, :], o_sb[:])
            # scale head: full G + softmax, replicated on every core
            for bt in range(2):
                o_sb = hout_pool.tile([128, G], f32, tag="o_sb", name="o_sb")
                for gh in range(2):
                    hps = hps_pool.tile([128, 512], f32, tag="hps", name="hps")
                    gsl = slice(2 * GS + gh * 512, 2 * GS + (gh + 1) * 512)
                    nc.tensor.matmul(
                        hps[:], ones1[:, 0:128], hb_sb[:, gsl],
                        start=True, stop=False,
                    )
                    nc.tensor.matmul(
                        hps[:],
                        hbn[:, bt * 128 : (bt + 1) * 128],
                        hw_sb[:, gsl],
                        start=False,
                        stop=True,
                    )
                    if gh == 0:
                        nc.scalar.copy(o_sb[:, gh * 512 : (gh + 1) * 512], hps[:])
                    else:
                        nc.vector.tensor_copy(
                            o_sb[:, gh * 512 : (gh + 1) * 512], hps[:]
                        )
                mx = hout_pool.tile([128, 1], f32, tag="mx", name="mx")
                nc.vector.tensor_reduce(mx[:], o_sb[:], axis=AX.X, op=ALU.max)
                nmx = hout_pool.tile([128, 1], f32, tag="nmx", name="nmx")
                nc.vector.tensor_scalar_mul(nmx[:], mx[:], -1.0)
                exl = hout_pool.tile([128, G], f32, tag="exl", name="exl")
                nc.scalar.activation(exl[:], o_sb[:], AF.Exp, bias=nmx[:])
                ssum = hout_pool.tile([128, 1], f32, tag="ssum", name="ssum")
                nc.vector.tensor_reduce(ssum[:], exl[:], axis=AX.X, op=ALU.add)
                sinv = hout_pool.tile([128, 1], f32, tag="sinv", name="sinv")
                nc.vector.reciprocal(sinv[:], ssum[:])
                smx = hout_pool.tile([128, G], f32, tag="smx", name="smx")
                nc.vector.tensor_scalar_mul(smx[:], exl[:], sinv[:])
                nc.sync.dma_start(psc_d[bt * 128 : (bt + 1) * 128, :], smx[:])

    nc.compile()
    _NC_CACHE[key] = nc
    return nc


_HOST_CACHE = {}


def _host_maps():
    """Static per-core index maps (depend only on shapes)."""
    if "maps" in _HOST_CACHE:
        return _HOST_CACHE["maps"]
    # per core r, per slot k: true i = 8k+r, valid len = G-1-i, local seg at SEG_OFF[k]
    maps = []
    for r in range(NCORES):
        rows = []  # (local_lo, local_hi, global_lo, global_hi)
        for k in range(NSLOT):
            i = NCORES * k + r
            vlen = G - 1 - i
            if vlen <= 0:
                continue
            o = _block_off(i)
            rows.append((SEG_OFF[k], SEG_OFF[k] + vlen, o, o + vlen))
        maps.append(rows)
    _HOST_CACHE["maps"] = maps
    return maps


def kernel(z, Wc, bc, W1, b1, gamma, beta, Wscale, bscale,
           Wshape, bshape, Wdrop, bdrop):
    return _run(z, Wc, bc, W1, b1, gamma, beta, Wscale, bscale,
                Wshape, bshape, Wdrop, bdrop)[0]


def _prep_in_maps(z, Wc, bc, W1, b1, gamma, beta, Wscale, bscale,
                  Wshape, bshape, Wdrop, bdrop):
    f32 = np.float32
    z = np.asarray(z, f32)
    Wc = np.asarray(Wc, f32)
    bc = np.asarray(bc, f32)
    W1 = np.asarray(W1, f32)
    gamma = np.asarray(gamma, f32)
    beta = np.asarray(beta, f32)
    maps = _host_maps()

    zT = np.ascontiguousarray(z.T)  # [NIN, B]
    WcT = np.ascontiguousarray(Wc.T)  # [NIN, G]
    W1T16 = np.ascontiguousarray(W1.T.astype(np.float16))  # [P, H]
    GS = G // NCORES
    wshT = np.ascontiguousarray(np.asarray(Wshape, f32).T)
    wdrT = np.ascontiguousarray(np.asarray(Wdrop, f32).T)
    bsh = np.asarray(bshape, f32).reshape(1, G)
    bdr = np.asarray(bdrop, f32).reshape(1, G)
    shared = {
        "zT": zT,
        "gamma": np.ascontiguousarray(gamma.reshape(H, 1)),
        "beta": np.ascontiguousarray(beta.reshape(H, 1)),
        "wscT": np.ascontiguousarray(np.asarray(Wscale, f32).T),
        "bsc": np.ascontiguousarray(np.asarray(bscale, f32).reshape(1, G)),
    }

    in_maps = []
    for r in range(NCORES):
        wcT_rot = np.zeros((NIN, G), f32)
        wcT_rot[:, : G - r] = WcT[:, r:]
        bc_rot = np.zeros((1, G), f32)
        bc_rot[0, : G - r] = bc[r:]
        w1t_r = np.zeros((PCPAD, H), np.float16)
        for (ll, lh, gl, gh) in maps[r]:
            w1t_r[ll:lh] = W1T16[gl:gh]
        # partition-major layout: [128, NPT*H], row p holds rows {t*128+p}
        w1t_r = np.ascontiguousarray(
            w1t_r.reshape(NPT, 128, H).transpose(1, 0, 2).reshape(128, NPT * H)
        )
        m = dict(shared)
        m["wcT_rot"] = wcT_rot
        m["bc_rot"] = bc_rot
        m["w1t"] = w1t_r
        m["wshT"] = np.ascontiguousarray(wshT[:, r * GS : (r + 1) * GS])
        m["wdrT"] = np.ascontiguousarray(wdrT[:, r * GS : (r + 1) * GS])
        m["bsh"] = np.ascontiguousarray(bsh[:, r * GS : (r + 1) * GS])
        m["bdr"] = np.ascontiguousarray(bdr[:, r * GS : (r + 1) * GS])
        in_maps.append(m)
    return in_maps


def _run(z, Wc, bc, W1, b1, gamma, beta, Wscale, bscale,
         Wshape, bshape, Wdrop, bdrop, trace=False, trace_cores=None):
    from concourse import bass_utils

    f32 = np.float32
    nc = _build_nc()
    maps = _host_maps()
    in_maps = _prep_in_maps(z, Wc, bc, W1, b1, gamma, beta, Wscale, bscale,
                            Wshape, bshape, Wdrop, bdrop)
    res = bass_utils.run_bass_kernel_spmd(
        nc, in_maps, core_ids=list(range(NCORES)),
        trace=trace, trace_cores=trace_cores,
    )
    outs = res.results

    px_sigma = np.empty((B, P), f32)
    for r in range(NCORES):
        sig_r = np.asarray(outs[r]["px_sig"], f32)
        for (ll, lh, gl, gh) in maps[r]:
            px_sigma[:, gl:gh] = sig_r[:, ll:lh]

    px_shape = np.concatenate([outs[r]["px_shape"] for r in range(NCORES)], axis=1)
    px_scale = outs[0]["px_scale"]
    px_dropout = np.concatenate([outs[r]["px_drop"] for r in range(NCORES)], axis=1)
    return (px_shape, px_scale, px_dropout, px_sigma), res


# revision 39
# speedup vs baseline: 777.8774x; 1.0067x over previous
"""Trainium2 Bass kernel for nn_DecoderVCSC (8-core SPMD).

Reference computation:
    c = z @ Wc.T + bc                                  (B, G)
    px_sigma = exp(-(c[:,iu] - c[:,jv]))               (B, P)   P = G*(G-1)/2
    h = px_sigma @ W1.T + b1                           (B, H)
    BN(training stats) + ReLU
    px_dropout = h @ Wdrop.T + bdrop
    px_shape   = h @ Wshape.T + bshape
    px_scale   = softmax(h @ Wscale.T + bscale)
    returns (px_shape, px_scale, px_dropout, px_sigma)

Sharding: the i<j pair blocks (block i = pairs (i, i+1..G-1)) are dealt
round-robin: core r handles blocks i = 8k + r, k = 0..127.  To keep the
SPMD instruction stream identical across cores, slot k is padded to
L_k = G-1-8k columns on every core (core r's true block length is
G-1-8k-r; the r pad columns are garbage on device and are (a) skipped by
the host unshard and (b) multiplied by host-zeroed W1T rows in the h
contraction).  The per-core shift r is absorbed on the host by passing a
rotated Wc (WcT_rot[:, t] = Wc[t+r, :]), so the device computes
cshift[b, t] = c[b, t+r] with core-independent addressing.

h is accumulated per-core over its p-slice and AllReduced across the 8
cores on-device; BN + heads + softmax run (redundantly) on every core.
b1 provably cancels in train-stats BN and is not sent to the device.
"""

import numpy as np

G = 1024
NIN = 64
H = 128
B = 256
P = G * (G - 1) // 2  # 523776
NCORES = 8
NSLOT = G // NCORES  # 128 slots per core
ALPHA = 1.0
BN_EPS = 1e-3

# Padded per-slot segment lengths/offsets (identical on all cores).
SEG_LEN = [G - 1 - NCORES * k for k in range(NSLOT)]  # 1023 - 8k
_off = np.cumsum([0] + SEG_LEN)
SEG_OFF = [int(x) for x in _off]
PCPAD = SEG_OFF[-1]  # 65920
NPT = PCPAD // 128  # 515 p-tiles of 128
assert PCPAD % 128 == 0

CHUNK = 2048  # sigma free-dim chunk width (elements)
CHUNKS = []
_c0 = 0
while _c0 < PCPAD:
    CHUNKS.append((_c0, min(_c0 + CHUNK, PCPAD)))
    _c0 += CHUNK


def _block_off(i):
    # global p offset of block i: sum_{t<i} (G-1-t)
    return i * (G - 1) - (i * (i - 1)) // 2


def _segments_in(lo, hi):
    """Yield (k, seg_lo, seg_hi) covering [lo,hi) split by slot boundaries."""
    out = []
    for k in range(NSLOT):
        s, e = SEG_OFF[k], SEG_OFF[k + 1]
        a, b = max(s, lo), min(e, hi)
        if a < b:
            out.append((k, a, b))
    return out


_NC_CACHE = {}


def _build_nc(single=False):
    key = "nc1" if single else "nc"
    if key in _NC_CACHE:
        return _NC_CACHE[key]
    import concourse.bass as bass
    import concourse.mybir as mybir
    import concourse.tile as tile
    from concourse import bacc
    from concourse.masks import make_identity

    f32 = mybir.dt.float32
    f32r = mybir.dt.float32r
    f16 = mybir.dt.float16
    AF = mybir.ActivationFunctionType
    ALU = mybir.AluOpType
    AX = mybir.AxisListType

    nc = bacc.Bacc(
        "TRN2", target_bir_lowering=False, debug=False,
        num_devices=1 if single else NCORES,
    )

    _pools = {}

    def _sgl(tc, shape, dtype, name, space="SBUF"):
        pool = _pools["psum" if space == "PSUM" else "const"]
        return pool.tile(shape, dtype, name=name, tag=name)

    # ---- I/O ----
    zT_d = nc.dram_tensor("zT", [NIN, B], f32r, kind="ExternalInput").ap()
    wcT_d = nc.dram_tensor("wcT_rot", [NIN, G], f32r, kind="ExternalInput").ap()
    bc_d = nc.dram_tensor("bc_rot", [1, G], f32r, kind="ExternalInput").ap()
    w1t_d = nc.dram_tensor("w1t", [128, NPT * H], f16, kind="ExternalInput").ap()
    gamma_d = nc.dram_tensor("gamma", [H, 1], f32, kind="ExternalInput").ap()
    beta_d = nc.dram_tensor("beta", [H, 1], f32, kind="ExternalInput").ap()
    GS = G // NCORES  # per-core head-output columns
    wscT_d = nc.dram_tensor("wscT", [H, G], f32r, kind="ExternalInput").ap()
    wshT_d = nc.dram_tensor("wshT", [H, GS], f32r, kind="ExternalInput").ap()
    wdrT_d = nc.dram_tensor("wdrT", [H, GS], f32r, kind="ExternalInput").ap()
    bsc_d = nc.dram_tensor("bsc", [1, G], f32r, kind="ExternalInput").ap()
    bsh_d = nc.dram_tensor("bsh", [1, GS], f32r, kind="ExternalInput").ap()
    bdr_d = nc.dram_tensor("bdr", [1, GS], f32r, kind="ExternalInput").ap()

    psig_d = nc.dram_tensor("px_sig", [B, PCPAD], f16, kind="ExternalOutput").ap()
    psh_d = nc.dram_tensor("px_shape", [B, GS], f32, kind="ExternalOutput").ap()
    psc_d = nc.dram_tensor("px_scale", [B, G], f32, kind="ExternalOutput").ap()
    pdr_d = nc.dram_tensor("px_drop", [B, GS], f32, kind="ExternalOutput").ap()

    from contextlib import ExitStack

    with tile.TileContext(nc) as tc, ExitStack() as _stack:
        _pools["const"] = _stack.enter_context(tc.tile_pool(name="const", bufs=1))
        _pools["psum"] = _stack.enter_context(
            tc.tile_pool(name="constps", bufs=1, space="PSUM")
        )
        # ---------- constants ----------
        identF = _sgl(tc, [128, 128], f32, name="identF")
        make_identity(nc, identF[:])
        ident = _sgl(tc, [128, 128], f16, name="ident")
        nc.vector.tensor_copy(ident[:], identF[:])
        identr = ident[:]
        ones1cf = _sgl(tc, [128, 128], f32, name="ones1cf")
        nc.vector.memset(ones1cf[:], 1.0)
        ones1c = ones1cf[0:1, :]
        ones1f = _sgl(tc, [128, 128], f32r, name="ones1f")
        nc.vector.tensor_copy(ones1f[:], ones1cf[:])
        ones1 = ones1f[0:1, :]

        # ---------- small input loads ----------
        zT_sbf = _sgl(tc, [128, B], f32r, name="zT_sbf")
        nc.sync.dma_start(zT_sbf[0:NIN, :], zT_d[:])
        zT_sb = zT_sbf[0:NIN, :]
        wcT_sbf = _sgl(tc, [128, G], f32r, name="wcT_sbf")
        nc.sync.dma_start(wcT_sbf[0:NIN, :], wcT_d[:])
        wcT_sb = wcT_sbf[0:NIN, :]
        bc_sbf = _sgl(tc, [128, G], f32r, name="bc_sbf")
        nc.sync.dma_start(bc_sbf[0:1, :], bc_d[:])
        bc_sb = bc_sbf[0:1, :]
        gamma_sb = _sgl(tc, [H, 1], f32, name="gamma_sb")
        nc.sync.dma_start(gamma_sb[:], gamma_d[:])
        beta_sb = _sgl(tc, [H, 1], f32, name="beta_sb")
        nc.sync.dma_start(beta_sb[:], beta_d[:])
        hw_sb = _sgl(tc, [H, G + 2 * GS], f32r, name="hw_sb")  # head weights (T)
        nc.sync.dma_start(hw_sb[:, 0:GS], wshT_d[:])
        nc.sync.dma_start(hw_sb[:, GS : 2 * GS], wdrT_d[:])
        nc.sync.dma_start(hw_sb[:, 2 * GS : 2 * GS + G], wscT_d[:])
        hb_sbf = _sgl(tc, [128, G + 2 * GS], f32r, name="hb_sbf")  # head biases
        hb_sb = hb_sbf[0:1, :]
        nc.sync.dma_start(hb_sbf[0:1, 0:GS], bsh_d[:])
        nc.sync.dma_start(hb_sbf[0:1, GS : 2 * GS], bdr_d[:])
        nc.sync.dma_start(hb_sbf[0:1, 2 * GS : 2 * GS + G], bsc_d[:])

        # ---------- cshift = (z @ WcT_rot) + bc_rot,  [2x128 b, 1024 g'] ----------
        ebuf = []  # exp(cshift)
        fbuf = []  # exp(-cshift)
        with tc.tile_pool(name="cpsum", bufs=4, space="PSUM") as cps_pool:
            for bt in range(2):
                e_sb = _sgl(tc, [128, G], f32, name=f"ebuf{bt}")
                f_sb = _sgl(tc, [128, G], f32, name=f"fbuf{bt}")
                for gh in range(2):  # two 512-wide psum tiles
                    cps = cps_pool.tile([128, 512], f32, tag="cps", name="cps")
                    gsl = slice(gh * 512, (gh + 1) * 512)
                    # bias broadcast via rank-1 matmul, then accumulate z@WcT
                    nc.tensor.matmul(
                        cps[:], ones1[:, 0:128], bc_sb[:, gsl], start=True, stop=False
                    )
                    nc.tensor.matmul(
                        cps[:],
                        zT_sb[:, bt * 128 : (bt + 1) * 128],
                        wcT_sb[:, gsl],
                        start=False,
                        stop=True,
                    )
                    nc.scalar.activation(e_sb[:, gsl], cps[:], AF.Exp)
                    nc.scalar.activation(f_sb[:, gsl], cps[:], AF.Exp, scale=-1.0)
                ebuf.append(e_sb)
                fbuf.append(f_sb)

        # ---------- main loop: sigma gen -> HBM + transpose -> h matmul ----------
        h_ps = _sgl(tc, [128, B], f32, name="h_ps", space="PSUM")  # h.T accumulator
        pt_global = 0
        with (
            tc.tile_pool(name="sig", bufs=4) as sig_pool,
            tc.tile_pool(name="sigT", bufs=6) as sigT_pool,
            tc.tile_pool(name="sigTps", bufs=6, space="PSUM") as sigTps_pool,
            tc.tile_pool(name="w1", bufs=12) as w1_pool,
        ):
            for (c0, c1) in CHUNKS:
                W = c1 - c0
                nt = W // 128
                # -- sigma generation (ACT), both b-tiles --
                sig = [
                    sig_pool.tile([128, W], f16, tag=f"sig{bt}", name=f"sig{bt}") for bt in range(2)
                ]
                for (k, a, b) in _segments_in(c0, c1):
                    # sigma[:, a:b] = exp(cshift[:, 8k+1+(a-S_k) : ...] - cshift[:, 8k])
                    j0 = NCORES * k + 1 + (a - SEG_OFF[k])
                    for bt in range(2):
                        nc.vector.tensor_scalar_mul(
                            sig[bt][:, a - c0 : b - c0],
                            ebuf[bt][:, j0 : j0 + (b - a)],
                            fbuf[bt][:, NCORES * k : NCORES * k + 1],
                        )
                # -- write sigma chunk to HBM --
                for bt in range(2):
                    nc.sync.dma_start(
                        psig_d[bt * 128 : (bt + 1) * 128, c0:c1], sig[bt][:]
                    )
                # -- W1T chunk load: [W, H] rows -> SBUF [128, nt*H] --
                w1c = w1_pool.tile([128, nt * H], f16, tag="w1c", name="w1c")
                nc.gpsimd.dma_start(
                    w1c[:], w1t_d[:, (c0 // 128) * H : (c1 // 128) * H]
                )
                # -- per p-tile: transpose sigma, accumulate h --
                for t2 in range(0, nt, 2):
                    npair = min(2, nt - t2)
                    tps = sigTps_pool.tile([128, 512], f16, tag="tps", name="tps")
                    for ti in range(npair):
                        for bt in range(2):
                            nc.tensor.transpose(
                                tps[:, ti * 256 + bt * 128 : ti * 256 + (bt + 1) * 128],
                                sig[bt][:, (t2 + ti) * 128 : (t2 + ti + 1) * 128],
                                identr[:],
                            )
                    sT = sigT_pool.tile([128, 512], f16, tag="sT", name="sT")
                    if (t2 // 2) % 5 == 0:
                        nc.vector.tensor_copy(
                            sT[:, : npair * 256], tps[:, : npair * 256]
                        )
                    else:
                        nc.scalar.copy(
                            sT[:, : npair * 256], tps[:, : npair * 256]
                        )
                    for ti in range(npair):
                        t = t2 + ti
                        nc.tensor.matmul(
                            h_ps[:],
                            w1c[:, t * H : (t + 1) * H],
                            sT[:, ti * 256 : (ti + 1) * 256],
                            start=(pt_global == 0),
                            stop=(pt_global == NPT - 1),
                            skip_group_check=True,
                        )
                        pt_global += 1

        # ---------- AllReduce h across cores ----------
        h_sb = _sgl(tc, [128, B], f32, name="h_sb")
        nc.scalar.copy(h_sb[:], h_ps[:])
        with tc.tile_pool(name="dram", bufs=2, space="DRAM") as dram:
            h_in = dram.tile([128, B], f32, name="hbounce")
            h_out = dram.tile([128, B], f32, name="hbounce")
            nc.sync.dma_start(h_in[:], h_sb[:])
            if single:
                # cost-model variant: stand in for the AllReduce with a copy
                nc.gpsimd.dma_start(h_out[:], h_in[:])
            else:
                nc.gpsimd.collective_compute(
                    "AllReduce",
                    ALU.add,
                    replica_groups=[list(range(NCORES))],
                    ins=[h_in[:].opt()],
                    outs=[h_out[:].opt()],
                )
            hall = _sgl(tc, [128, B], f32, name="hall")
            nc.sync.dma_start(hall[:], h_out[:])

        # ---------- BatchNorm (training stats) + ReLU, in [k, b] layout ----------
        musum = _sgl(tc, [128, 1], f32, name="musum")
        nc.vector.tensor_reduce(musum[:], hall[:], axis=AX.X, op=ALU.add)
        hsq = _sgl(tc, [128, B], f32, name="hsq")
        nc.vector.tensor_tensor(hsq[:], hall[:], hall[:], op=ALU.mult)
        sqsum = _sgl(tc, [128, 1], f32, name="sqsum")
        nc.vector.tensor_reduce(sqsum[:], hsq[:], axis=AX.X, op=ALU.add)
        mu = _sgl(tc, [128, 1], f32, name="mu")
        nc.vector.tensor_scalar_mul(mu[:], musum[:], 1.0 / B)
        musq = _sgl(tc, [128, 1], f32, name="musq")
        nc.vector.tensor_tensor(musq[:], mu[:], mu[:], op=ALU.mult)
        var = _sgl(tc, [128, 1], f32, name="var")
        nc.vector.scalar_tensor_tensor(
            var[:], sqsum[:], 1.0 / B, musq[:],
            op0=ALU.mult, op1=ALU.subtract,
        )
        varp = _sgl(tc, [128, 1], f32, name="varp")
        nc.vector.tensor_scalar_add(varp[:], var[:], BN_EPS)
        i32 = mybir.dt.int32
        magic = _sgl(tc, [128, 1], i32, name="magic")
        nc.vector.memset(magic[:], 0x5F3759DF)
        ihalf = _sgl(tc, [128, 1], i32, name="ihalf")
        nc.vector.tensor_scalar(
            ihalf[:], varp[:].bitcast(i32), 1, None, op0=ALU.arith_shift_right
        )
        yint = _sgl(tc, [128, 1], i32, name="yint")
        nc.vector.tensor_tensor(yint[:], magic[:], ihalf[:], op=ALU.subtract)
        rst = yint[:].bitcast(f32)
        for _it in range(3):
            nt1 = _sgl(tc, [128, 1], f32, name=f"nt1_{_it}")
            nc.vector.tensor_tensor(nt1[:], varp[:], rst, op=ALU.mult)
            nt2 = _sgl(tc, [128, 1], f32, name=f"nt2_{_it}")
            nc.vector.tensor_tensor(nt2[:], nt1[:], rst, op=ALU.mult)
            nt3 = _sgl(tc, [128, 1], f32, name=f"nt3_{_it}")
            nc.vector.tensor_scalar(
                nt3[:], nt2[:], -0.5, 1.5, op0=ALU.mult, op1=ALU.add
            )
            nt4 = _sgl(tc, [128, 1], f32, name=f"nt4_{_it}")
            nc.vector.tensor_tensor(nt4[:], nt3[:], rst, op=ALU.mult)
            rst = nt4[:]
        bnsc = _sgl(tc, [128, 1], f32, name="bnsc")
        nc.vector.tensor_tensor(bnsc[:], gamma_sb[:], rst, op=ALU.mult)
        mbs = _sgl(tc, [128, 1], f32, name="mbs")
        nc.vector.tensor_tensor(mbs[:], mu[:], bnsc[:], op=ALU.mult)
        bnbi = _sgl(tc, [128, 1], f32, name="bnbi")
        nc.vector.tensor_tensor(bnbi[:], beta_sb[:], mbs[:], op=ALU.subtract)
        hbn = _sgl(tc, [128, B], f32r, name="hbn")
        nc.scalar.activation(hbn[:], hall[:], AF.Relu, bias=bnbi[:], scale=bnsc[:])

        # ---------- heads ----------
        # sharded shape/drop: this core computes its GS output columns
        with (
            tc.tile_pool(name="hps", bufs=3, space="PSUM") as hps_pool,
            tc.tile_pool(name="hout", bufs=4) as hout_pool,
        ):
            for hd, (out_d, ev_eng) in enumerate(
                [(psh_d, nc.scalar), (pdr_d, nc.vector)]
            ):
                off = hd * GS
                for bt in range(2):
                    hps = hps_pool.tile([128, 512], f32, tag="hps", name="hps")[:, 0:GS]
                    nc.tensor.matmul(
                        hps[:], ones1[:, 0:128], hb_sb[:, off : off + GS],
                        start=True, stop=False,
                    )
                    nc.tensor.matmul(
                        hps[:],
                        hbn[:, bt * 128 : (bt + 1) * 128],
                        hw_sb[:, off : off + GS],
                        start=False,
                        stop=True,
                    )
                    o_sb = hout_pool.tile([128, GS], f32, tag="osm", name="o_sb")
                    if hd == 0:
                        ev_eng.copy(o_sb[:], hps[:])
                    else:
                        ev_eng.tensor_copy(o_sb[:], hps[:])
                    nc.sync.dma_start(out_d[bt * 128 : (bt + 1) * 128, :], o_sb[:])
            # scale head: full G + softmax, replicated on every core
            for bt in range(2):
                o_sb = hout_pool.tile([128, G], f32, tag="o_sb", name="o_sb")
                for gh in range(2):
                    hps = hps_pool.tile([128, 512], f32, tag="hps", name="hps")
                    gsl = slice(2 * GS + gh * 512, 2 * GS + (gh + 1) * 512)
                    nc.tensor.matmul(
                        hps[:], ones1[:, 0:128], hb_sb[:, gsl],
                        start=True, stop=False,
                    )
                    nc.tensor.matmul(
                        hps[:],
                        hbn[:, bt * 128 : (bt + 1) * 128],
                        hw_sb[:, gsl],
                        start=False,
                        stop=True,
                    )
                    if gh == 0:
                        nc.scalar.copy(o_sb[:, gh * 512 : (gh + 1) * 512], hps[:])
                    else:
                        nc.vector.tensor_copy(
                            o_sb[:, gh * 512 : (gh + 1) * 512], hps[:]
                        )
                mx = hout_pool.tile([128, 1], f32, tag="mx", name="mx")
                nc.vector.tensor_reduce(mx[:], o_sb[:], axis=AX.X, op=ALU.max)
                nmx = hout_pool.tile([128, 1], f32, tag="nmx", name="nmx")
                nc.vector.tensor_scalar_mul(nmx[:], mx[:], -1.0)
                exl = hout_pool.tile([128, G], f32, tag="exl", name="exl")
                nc.scalar.activation(exl[:], o_sb[:], AF.Exp, bias=nmx[:])
                ssum = hout_pool.tile([128, 1], f32, tag="ssum", name="ssum")
                nc.vector.tensor_reduce(ssum[:], exl[:], axis=AX.X, op=ALU.add)
                sinv = hout_pool.tile([128, 1], f32, tag="sinv", name="sinv")
                nc.vector.reciprocal(sinv[:], ssum[:])
                smx = hout_pool.tile([128, G], f32, tag="smx", name="smx")
                nc.vector.tensor_scalar_mul(smx[:], exl[:], sinv[:])
                nc.sync.dma_start(psc_d[bt * 128 : (bt + 1) * 128, :], smx[:])

    nc.compile()
    _NC_CACHE[key] = nc
    return nc


_HOST_CACHE = {}


def _host_maps():
    """Static per-core index maps (depend only on shapes)."""
    if "maps" in _HOST_CACHE:
        return _HOST_CACHE["maps"]
    # per core r, per slot k: true i = 8k+r, valid len = G-1-i, local seg at SEG_OFF[k]
    maps = []
    for r in range(NCORES):
        rows = []  # (local_lo, local_hi, global_lo, global_hi)
        for k in range(NSLOT):
            i = NCORES * k + r
            vlen = G - 1 - i
            if vlen <= 0:
                continue
            o = _block_off(i)
            rows.append((SEG_OFF[k], SEG_OFF[k] + vlen, o, o + vlen))
        maps.append(rows)
    _HOST_CACHE["maps"] = maps
    return maps


def kernel(z, Wc, bc, W1, b1, gamma, beta, Wscale, bscale,
           Wshape, bshape, Wdrop, bdrop):
    return _run(z, Wc, bc, W1, b1, gamma, beta, Wscale, bscale,
                Wshape, bshape, Wdrop, bdrop)[0]


def _prep_in_maps(z, Wc, bc, W1, b1, gamma, beta, Wscale, bscale,
                  Wshape, bshape, Wdrop, bdrop):
    f32 = np.float32
    z = np.asarray(z, f32)
    Wc = np.asarray(Wc, f32)
    bc = np.asarray(bc, f32)
    W1 = np.asarray(W1, f32)
    gamma = np.asarray(gamma, f32)
    beta = np.asarray(beta, f32)
    maps = _host_maps()

    zT = np.ascontiguousarray(z.T)  # [NIN, B]
    WcT = np.ascontiguousarray(Wc.T)  # [NIN, G]
    W1T16 = np.ascontiguousarray(W1.T.astype(np.float16))  # [P, H]
    GS = G // NCORES
    wshT = np.ascontiguousarray(np.asarray(Wshape, f32).T)
    wdrT = np.ascontiguousarray(np.asarray(Wdrop, f32).T)
    bsh = np.asarray(bshape, f32).reshape(1, G)
    bdr = np.asarray(bdrop, f32).reshape(1, G)
    shared = {
        "zT": zT,
        "gamma": np.ascontiguousarray(gamma.reshape(H, 1)),
        "beta": np.ascontiguousarray(beta.reshape(H, 1)),
        "wscT": np.ascontiguousarray(np.asarray(Wscale, f32).T),
        "bsc": np.ascontiguousarray(np.asarray(bscale, f32).reshape(1, G)),
    }

    in_maps = []
    for r in range(NCORES):
        wcT_rot = np.zeros((NIN, G), f32)
        wcT_rot[:, : G - r] = WcT[:, r:]
        bc_rot = np.zeros((1, G), f32)
        bc_rot[0, : G - r] = bc[r:]
        w1t_r = np.zeros((PCPAD, H), np.float16)
        for (ll, lh, gl, gh) in maps[r]:
            w1t_r[ll:lh] = W1T16[gl:gh]
        # partition-major layout: [128, NPT*H], row p holds rows {t*128+p}
        w1t_r = np.ascontiguousarray(
            w1t_r.reshape(NPT, 128, H).transpose(1, 0, 2).reshape(128, NPT * H)
        )
        m = dict(shared)
        m["wcT_rot"] = wcT_rot
        m["bc_rot"] = bc_rot
        m["w1t"] = w1t_r
        m["wshT"] = np.ascontiguousarray(wshT[:, r * GS : (r + 1) * GS])
        m["wdrT"] = np.ascontiguousarray(wdrT[:, r * GS : (r + 1) * GS])
        m["bsh"] = np.ascontiguousarray(bsh[:, r * GS : (r + 1) * GS])
        m["bdr"] = np.ascontiguousarray(bdr[:, r * GS : (r + 1) * GS])
        in_maps.append(m)
    return in_maps


def _run(z, Wc, bc, W1, b1, gamma, beta, Wscale, bscale,
         Wshape, bshape, Wdrop, bdrop, trace=False, trace_cores=None):
    from concourse import bass_utils

    f32 = np.float32
    nc = _build_nc()
    maps = _host_maps()
    in_maps = _prep_in_maps(z, Wc, bc, W1, b1, gamma, beta, Wscale, bscale,
                            Wshape, bshape, Wdrop, bdrop)
    res = bass_utils.run_bass_kernel_spmd(
        nc, in_maps, core_ids=list(range(NCORES)),
        trace=trace, trace_cores=trace_cores,
    )
    outs = res.results

    px_sigma = np.empty((B, P), f32)
    for r in range(NCORES):
        sig_r = np.asarray(outs[r]["px_sig"], f32)
        for (ll, lh, gl, gh) in maps[r]:
            px_sigma[:, gl:gh] = sig_r[:, ll:lh]

    px_shape = np.concatenate([outs[r]["px_shape"] for r in range(NCORES)], axis=1)
    px_scale = outs[0]["px_scale"]
    px_dropout = np.concatenate([outs[r]["px_drop"] for r in range(NCORES)], axis=1)
    return (px_shape, px_scale, px_dropout, px_sigma), res


# revision 45
# speedup vs baseline: 779.5180x; 1.0021x over previous
"""Trainium2 Bass kernel for nn_DecoderVCSC (8-core SPMD).

Reference computation:
    c = z @ Wc.T + bc                                  (B, G)
    px_sigma = exp(-(c[:,iu] - c[:,jv]))               (B, P)   P = G*(G-1)/2
    h = px_sigma @ W1.T + b1                           (B, H)
    BN(training stats) + ReLU
    px_dropout = h @ Wdrop.T + bdrop
    px_shape   = h @ Wshape.T + bshape
    px_scale   = softmax(h @ Wscale.T + bscale)
    returns (px_shape, px_scale, px_dropout, px_sigma)

Sharding: the i<j pair blocks (block i = pairs (i, i+1..G-1)) are dealt
round-robin: core r handles blocks i = 8k + r, k = 0..127.  To keep the
SPMD instruction stream identical across cores, slot k is padded to
L_k = G-1-8k columns on every core (core r's true block length is
G-1-8k-r; the r pad columns are garbage on device and are (a) skipped by
the host unshard and (b) multiplied by host-zeroed W1T rows in the h
contraction).  The per-core shift r is absorbed on the host by passing a
rotated Wc (WcT_rot[:, t] = Wc[t+r, :]), so the device computes
cshift[b, t] = c[b, t+r] with core-independent addressing.

h is accumulated per-core over its p-slice and AllReduced across the 8
cores on-device; BN + heads + softmax run (redundantly) on every core.
b1 provably cancels in train-stats BN and is not sent to the device.
"""

import numpy as np

G = 1024
NIN = 64
H = 128
B = 256
P = G * (G - 1) // 2  # 523776
NCORES = 8
NSLOT = G // NCORES  # 128 slots per core
ALPHA = 1.0
BN_EPS = 1e-3

# Padded per-slot segment lengths/offsets (identical on all cores).
SEG_LEN = [G - 1 - NCORES * k for k in range(NSLOT)]  # 1023 - 8k
_off = np.cumsum([0] + SEG_LEN)
SEG_OFF = [int(x) for x in _off]
PCPAD = SEG_OFF[-1]  # 65920
NPT = PCPAD // 128  # 515 p-tiles of 128
assert PCPAD % 128 == 0

CHUNK = 2048  # sigma free-dim chunk width (elements)
CHUNKS = []
_c0 = 0
while _c0 < PCPAD:
    CHUNKS.append((_c0, min(_c0 + CHUNK, PCPAD)))
    _c0 += CHUNK


def _block_off(i):
    # global p offset of block i: sum_{t<i} (G-1-t)
    return i * (G - 1) - (i * (i - 1)) // 2


def _segments_in(lo, hi):
    """Yield (k, seg_lo, seg_hi) covering [lo,hi) split by slot boundaries."""
    out = []
    for k in range(NSLOT):
        s, e = SEG_OFF[k], SEG_OFF[k + 1]
        a, b = max(s, lo), min(e, hi)
        if a < b:
            out.append((k, a, b))
    return out


_NC_CACHE = {}


def _build_nc(single=False):
    key = "nc1" if single else "nc"
    if key in _NC_CACHE:
        return _NC_CACHE[key]
    import concourse.bass as bass
    import concourse.mybir as mybir
    import concourse.tile as tile
    from concourse import bacc
    from concourse.masks import make_identity

    f32 = mybir.dt.float32
    f32r = mybir.dt.float32r
    f16 = mybir.dt.float16
    AF = mybir.ActivationFunctionType
    ALU = mybir.AluOpType
    AX = mybir.AxisListType

    nc = bacc.Bacc(
        "TRN2", target_bir_lowering=False, debug=False,
        num_devices=1 if single else NCORES,
    )

    _pools = {}

    def _sgl(tc, shape, dtype, name, space="SBUF"):
        pool = _pools["psum" if space == "PSUM" else "const"]
        return pool.tile(shape, dtype, name=name, tag=name)

    # ---- I/O ----
    zT_d = nc.dram_tensor("zT", [NIN, B], f32r, kind="ExternalInput").ap()
    wcT_d = nc.dram_tensor("wcT_rot", [NIN, G], f32r, kind="ExternalInput").ap()
    bc_d = nc.dram_tensor("bc_rot", [1, G], f32r, kind="ExternalInput").ap()
    w1t_d = nc.dram_tensor("w1t", [128, NPT * H], f16, kind="ExternalInput").ap()
    gamma_d = nc.dram_tensor("gamma", [H, 1], f32, kind="ExternalInput").ap()
    beta_d = nc.dram_tensor("beta", [H, 1], f32, kind="ExternalInput").ap()
    GS = G // NCORES  # per-core head-output columns
    wscT_d = nc.dram_tensor("wscT", [H, G], f32r, kind="ExternalInput").ap()
    wshT_d = nc.dram_tensor("wshT", [H, GS], f32r, kind="ExternalInput").ap()
    wdrT_d = nc.dram_tensor("wdrT", [H, GS], f32r, kind="ExternalInput").ap()
    bsc_d = nc.dram_tensor("bsc", [1, G], f32r, kind="ExternalInput").ap()
    bsh_d = nc.dram_tensor("bsh", [1, GS], f32r, kind="ExternalInput").ap()
    bdr_d = nc.dram_tensor("bdr", [1, GS], f32r, kind="ExternalInput").ap()

    psig_d = nc.dram_tensor("px_sig", [B, PCPAD], f16, kind="ExternalOutput").ap()
    psh_d = nc.dram_tensor("px_shape", [B, GS], f32, kind="ExternalOutput").ap()
    psc_d = nc.dram_tensor("px_scale", [B, G], f32, kind="ExternalOutput").ap()
    pdr_d = nc.dram_tensor("px_drop", [B, GS], f32, kind="ExternalOutput").ap()

    from contextlib import ExitStack

    with tile.TileContext(nc) as tc, ExitStack() as _stack:
        _pools["const"] = _stack.enter_context(tc.tile_pool(name="const", bufs=1))
        _pools["psum"] = _stack.enter_context(
            tc.tile_pool(name="constps", bufs=1, space="PSUM")
        )
        # ---------- constants ----------
        identF = _sgl(tc, [128, 128], f32, name="identF")
        make_identity(nc, identF[:])
        ident = _sgl(tc, [128, 128], f16, name="ident")
        nc.vector.tensor_copy(ident[:], identF[:])
        identr = ident[:]
        ones1cf = _sgl(tc, [128, 128], f32, name="ones1cf")
        nc.vector.memset(ones1cf[:], 1.0)
        ones1c = ones1cf[0:1, :]
        ones1f = _sgl(tc, [128, 128], f32r, name="ones1f")
        nc.vector.tensor_copy(ones1f[:], ones1cf[:])
        ones1 = ones1f[0:1, :]

        # ---------- small input loads ----------
        zT_sbf = _sgl(tc, [128, B], f32r, name="zT_sbf")
        nc.sync.dma_start(zT_sbf[0:NIN, :], zT_d[:])
        zT_sb = zT_sbf[0:NIN, :]
        wcT_sbf = _sgl(tc, [128, G], f32r, name="wcT_sbf")
        nc.sync.dma_start(wcT_sbf[0:NIN, :], wcT_d[:])
        wcT_sb = wcT_sbf[0:NIN, :]
        bc_sbf = _sgl(tc, [128, G], f32r, name="bc_sbf")
        nc.sync.dma_start(bc_sbf[0:1, :], bc_d[:])
        bc_sb = bc_sbf[0:1, :]
        gamma_sb = _sgl(tc, [H, 1], f32, name="gamma_sb")
        nc.sync.dma_start(gamma_sb[:], gamma_d[:])
        beta_sb = _sgl(tc, [H, 1], f32, name="beta_sb")
        nc.sync.dma_start(beta_sb[:], beta_d[:])
        hw_sb = _sgl(tc, [H, G + 2 * GS], f32r, name="hw_sb")  # head weights (T)
        nc.sync.dma_start(hw_sb[:, 0:GS], wshT_d[:])
        nc.sync.dma_start(hw_sb[:, GS : 2 * GS], wdrT_d[:])
        nc.sync.dma_start(hw_sb[:, 2 * GS : 2 * GS + G], wscT_d[:])
        hb_sbf = _sgl(tc, [128, G + 2 * GS], f32r, name="hb_sbf")  # head biases
        hb_sb = hb_sbf[0:1, :]
        nc.sync.dma_start(hb_sbf[0:1, 0:GS], bsh_d[:])
        nc.sync.dma_start(hb_sbf[0:1, GS : 2 * GS], bdr_d[:])
        nc.sync.dma_start(hb_sbf[0:1, 2 * GS : 2 * GS + G], bsc_d[:])

        # ---------- cshift = (z @ WcT_rot) + bc_rot,  [2x128 b, 1024 g'] ----------
        ebuf = []  # exp(cshift)
        fbuf = []  # exp(-cshift)
        with tc.tile_pool(name="cpsum", bufs=4, space="PSUM") as cps_pool:
            for bt in range(2):
                e_sb = _sgl(tc, [128, G], f32, name=f"ebuf{bt}")
                f_sb = _sgl(tc, [128, G], f32, name=f"fbuf{bt}")
                for gh in range(2):  # two 512-wide psum tiles
                    cps = cps_pool.tile([128, 512], f32, tag="cps", name="cps")
                    gsl = slice(gh * 512, (gh + 1) * 512)
                    # bias broadcast via rank-1 matmul, then accumulate z@WcT
                    nc.tensor.matmul(
                        cps[:], ones1[:, 0:128], bc_sb[:, gsl], start=True, stop=False
                    )
                    nc.tensor.matmul(
                        cps[:],
                        zT_sb[:, bt * 128 : (bt + 1) * 128],
                        wcT_sb[:, gsl],
                        start=False,
                        stop=True,
                    )
                    nc.scalar.activation(e_sb[:, gsl], cps[:], AF.Exp)
                    nc.scalar.activation(f_sb[:, gsl], cps[:], AF.Exp, scale=-1.0)
                ebuf.append(e_sb)
                fbuf.append(f_sb)

        # ---------- main loop: sigma gen -> HBM + transpose -> h matmul ----------
        h_ps = _sgl(tc, [128, B], f32, name="h_ps", space="PSUM")  # h.T accumulator
        pt_global = 0
        with (
            tc.tile_pool(name="sig", bufs=4) as sig_pool,
            tc.tile_pool(name="sigT", bufs=8) as sigT_pool,
            tc.tile_pool(name="sigTps", bufs=7, space="PSUM") as sigTps_pool,
            tc.tile_pool(name="w1", bufs=12) as w1_pool,
        ):
            for (c0, c1) in CHUNKS:
                W = c1 - c0
                nt = W // 128
                # -- sigma generation (ACT), both b-tiles --
                sig = [
                    sig_pool.tile([128, W], f16, tag=f"sig{bt}", name=f"sig{bt}") for bt in range(2)
                ]
                for (k, a, b) in _segments_in(c0, c1):
                    # sigma[:, a:b] = exp(cshift[:, 8k+1+(a-S_k) : ...] - cshift[:, 8k])
                    j0 = NCORES * k + 1 + (a - SEG_OFF[k])
                    for bt in range(2):
                        nc.vector.tensor_scalar_mul(
                            sig[bt][:, a - c0 : b - c0],
                            ebuf[bt][:, j0 : j0 + (b - a)],
                            fbuf[bt][:, NCORES * k : NCORES * k + 1],
                        )
                # -- write sigma chunk to HBM --
                for bt in range(2):
                    nc.sync.dma_start(
                        psig_d[bt * 128 : (bt + 1) * 128, c0:c1], sig[bt][:]
                    )
                # -- W1T chunk load: [W, H] rows -> SBUF [128, nt*H] --
                w1c = w1_pool.tile([128, nt * H], f16, tag="w1c", name="w1c")
                nc.gpsimd.dma_start(
                    w1c[:], w1t_d[:, (c0 // 128) * H : (c1 // 128) * H]
                )
                # -- per p-tile: transpose sigma, accumulate h --
                for t2 in range(0, nt, 2):
                    npair = min(2, nt - t2)
                    tps = sigTps_pool.tile([128, 512], f16, tag="tps", name="tps")
                    for ti in range(npair):
                        for bt in range(2):
                            nc.tensor.transpose(
                                tps[:, ti * 256 + bt * 128 : ti * 256 + (bt + 1) * 128],
                                sig[bt][:, (t2 + ti) * 128 : (t2 + ti + 1) * 128],
                                identr[:],
                            )
                    sT = sigT_pool.tile([128, 512], f16, tag="sT", name="sT")
                    if (t2 // 2) % 5 == 0:
                        nc.vector.tensor_copy(
                            sT[:, : npair * 256], tps[:, : npair * 256]
                        )
                    else:
                        nc.scalar.copy(
                            sT[:, : npair * 256], tps[:, : npair * 256]
                        )
                    for ti in range(npair):
                        t = t2 + ti
                        nc.tensor.matmul(
                            h_ps[:],
                            w1c[:, t * H : (t + 1) * H],
                            sT[:, ti * 256 : (ti + 1) * 256],
                            start=(pt_global == 0),
                            stop=(pt_global == NPT - 1),
                            skip_group_check=True,
                        )
                        pt_global += 1

        # ---------- AllReduce h across cores ----------
        h_sb = _sgl(tc, [128, B], f32, name="h_sb")
        nc.scalar.copy(h_sb[:], h_ps[:])
        with tc.tile_pool(name="dram", bufs=2, space="DRAM") as dram:
            h_in = dram.tile([128, B], f32, name="hbounce")
            h_out = dram.tile([128, B], f32, name="hbounce")
            nc.sync.dma_start(h_in[:], h_sb[:])
            if single:
                # cost-model variant: stand in for the AllReduce with a copy
                nc.gpsimd.dma_start(h_out[:], h_in[:])
            else:
                nc.gpsimd.collective_compute(
                    "AllReduce",
                    ALU.add,
                    replica_groups=[list(range(NCORES))],
                    ins=[h_in[:].opt()],
                    outs=[h_out[:].opt()],
                )
            hall = _sgl(tc, [128, B], f32, name="hall")
            nc.sync.dma_start(hall[:], h_out[:])

        # ---------- BatchNorm (training stats) + ReLU, in [k, b] layout ----------
        musum = _sgl(tc, [128, 1], f32, name="musum")
        nc.vector.tensor_reduce(musum[:], hall[:], axis=AX.X, op=ALU.add)
        hsq = _sgl(tc, [128, B], f32, name="hsq")
        nc.vector.tensor_tensor(hsq[:], hall[:], hall[:], op=ALU.mult)
        sqsum = _sgl(tc, [128, 1], f32, name="sqsum")
        nc.vector.tensor_reduce(sqsum[:], hsq[:], axis=AX.X, op=ALU.add)
        mu = _sgl(tc, [128, 1], f32, name="mu")
        nc.vector.tensor_scalar_mul(mu[:], musum[:], 1.0 / B)
        musq = _sgl(tc, [128, 1], f32, name="musq")
        nc.vector.tensor_tensor(musq[:], mu[:], mu[:], op=ALU.mult)
        var = _sgl(tc, [128, 1], f32, name="var")
        nc.vector.scalar_tensor_tensor(
            var[:], sqsum[:], 1.0 / B, musq[:],
            op0=ALU.mult, op1=ALU.subtract,
        )
        varp = _sgl(tc, [128, 1], f32, name="varp")
        nc.vector.tensor_scalar_add(varp[:], var[:], BN_EPS)
        i32 = mybir.dt.int32
        magic = _sgl(tc, [128, 1], i32, name="magic")
        nc.vector.memset(magic[:], 0x5F3759DF)
        ihalf = _sgl(tc, [128, 1], i32, name="ihalf")
        nc.vector.tensor_scalar(
            ihalf[:], varp[:].bitcast(i32), 1, None, op0=ALU.arith_shift_right
        )
        yint = _sgl(tc, [128, 1], i32, name="yint")
        nc.vector.tensor_tensor(yint[:], magic[:], ihalf[:], op=ALU.subtract)
        rst = yint[:].bitcast(f32)
        for _it in range(3):
            nt1 = _sgl(tc, [128, 1], f32, name=f"nt1_{_it}")
            nc.vector.tensor_tensor(nt1[:], varp[:], rst, op=ALU.mult)
            nt2 = _sgl(tc, [128, 1], f32, name=f"nt2_{_it}")
            nc.vector.tensor_tensor(nt2[:], nt1[:], rst, op=ALU.mult)
            nt3 = _sgl(tc, [128, 1], f32, name=f"nt3_{_it}")
            nc.vector.tensor_scalar(
                nt3[:], nt2[:], -0.5, 1.5, op0=ALU.mult, op1=ALU.add
            )
            nt4 = _sgl(tc, [128, 1], f32, name=f"nt4_{_it}")
            nc.vector.tensor_tensor(nt4[:], nt3[:], rst, op=ALU.mult)
            rst = nt4[:]
        bnsc = _sgl(tc, [128, 1], f32, name="bnsc")
        nc.vector.tensor_tensor(bnsc[:], gamma_sb[:], rst, op=ALU.mult)
        mbs = _sgl(tc, [128, 1], f32, name="mbs")
        nc.vector.tensor_tensor(mbs[:], mu[:], bnsc[:], op=ALU.mult)
        bnbi = _sgl(tc, [128, 1], f32, name="bnbi")
        nc.vector.tensor_tensor(bnbi[:], beta_sb[:], mbs[:], op=ALU.subtract)
        hbn = _sgl(tc, [128, B], f32r, name="hbn")
        nc.scalar.activation(hbn[:], hall[:], AF.Relu, bias=bnbi[:], scale=bnsc[:])

        # ---------- heads ----------
        # sharded shape/drop: this core computes its GS output columns
        with (
            tc.tile_pool(name="hps", bufs=3, space="PSUM") as hps_pool,
            tc.tile_pool(name="hout", bufs=4) as hout_pool,
        ):
            for hd, (out_d, ev_eng) in enumerate(
                [(psh_d, nc.scalar), (pdr_d, nc.vector)]
            ):
                off = hd * GS
                for bt in range(2):
                    hps = hps_pool.tile([128, 512], f32, tag="hps", name="hps")[:, 0:GS]
                    nc.tensor.matmul(
                        hps[:], ones1[:, 0:128], hb_sb[:, off : off + GS],
                        start=True, stop=False,
                    )
                    nc.tensor.matmul(
                        hps[:],
                        hbn[:, bt * 128 : (bt + 1) * 128],
                        hw_sb[:, off : off + GS],
                        start=False,
                        stop=True,
                    )
                    o_sb = hout_pool.tile([128, GS], f32, tag="osm", name="o_sb")
                    if hd == 0:
                        ev_eng.copy(o_sb[:], hps[:])
                    else:
                        ev_eng.tensor_copy(o_sb[:], hps[:])
                    nc.sync.dma_start(out_d[bt * 128 : (bt + 1) * 128, :], o_sb[:])
            # scale head: full G + softmax, replicated on every core
            for bt in range(2):
                o_sb = hout_pool.tile([128, G], f32, tag="o_sb", name="o_sb")
                for gh in range(2):
                    hps = hps_pool.tile([128, 512], f32, tag="hps", name="hps")
                    gsl = slice(2 * GS + gh * 512, 2 * GS + (gh + 1) * 512)
                    nc.tensor.matmul(
                        hps[:], ones1[:, 0:128], hb_sb[:, gsl],
                        start=True, stop=False,
                    )
                    nc.tensor.matmul(
                        hps[:],
                        hbn[:, bt * 128 : (bt + 1) * 128],
                        hw_sb[:, gsl],
                        start=False,
                        stop=True,
                    )
                    if gh == 0:
                        nc.scalar.copy(o_sb[:, gh * 512 : (gh + 1) * 512], hps[:])
                    else:
                        nc.vector.tensor_copy(
                            o_sb[:, gh * 512 : (gh + 1) * 512], hps[:]
                        )
                mx = hout_pool.tile([128, 1], f32, tag="mx", name="mx")
                nc.vector.tensor_reduce(mx[:], o_sb[:], axis=AX.X, op=ALU.max)
                nmx = hout_pool.tile([128, 1], f32, tag="nmx", name="nmx")
                nc.vector.tensor_scalar_mul(nmx[:], mx[:], -1.0)
                exl = hout_pool.tile([128, G], f32, tag="exl", name="exl")
                nc.scalar.activation(exl[:], o_sb[:], AF.Exp, bias=nmx[:])
                ssum = hout_pool.tile([128, 1], f32, tag="ssum", name="ssum")
                nc.vector.tensor_reduce(ssum[:], exl[:], axis=AX.X, op=ALU.add)
                sinv = hout_pool.tile([128, 1], f32, tag="sinv", name="sinv")
                nc.vector.reciprocal(sinv[:], ssum[:])
                smx = hout_pool.tile([128, G], f32, tag="smx", name="smx")
                nc.vector.tensor_scalar_mul(smx[:], exl[:], sinv[:])
                nc.sync.dma_start(psc_d[bt * 128 : (bt + 1) * 128, :], smx[:])

    nc.compile()
    _NC_CACHE[key] = nc
    return nc


_HOST_CACHE = {}


def _host_maps():
    """Static per-core index maps (depend only on shapes)."""
    if "maps" in _HOST_CACHE:
        return _HOST_CACHE["maps"]
    # per core r, per slot k: true i = 8k+r, valid len = G-1-i, local seg at SEG_OFF[k]
    maps = []
    for r in range(NCORES):
        rows = []  # (local_lo, local_hi, global_lo, global_hi)
        for k in range(NSLOT):
            i = NCORES * k + r
            vlen = G - 1 - i
            if vlen <= 0:
                continue
            o = _block_off(i)
            rows.append((SEG_OFF[k], SEG_OFF[k] + vlen, o, o + vlen))
        maps.append(rows)
    _HOST_CACHE["maps"] = maps
    return maps


def kernel(z, Wc, bc, W1, b1, gamma, beta, Wscale, bscale,
           Wshape, bshape, Wdrop, bdrop):
    return _run(z, Wc, bc, W1, b1, gamma, beta, Wscale, bscale,
                Wshape, bshape, Wdrop, bdrop)[0]


def _prep_in_maps(z, Wc, bc, W1, b1, gamma, beta, Wscale, bscale,
                  Wshape, bshape, Wdrop, bdrop):
    f32 = np.float32
    z = np.asarray(z, f32)
    Wc = np.asarray(Wc, f32)
    bc = np.asarray(bc, f32)
    W1 = np.asarray(W1, f32)
    gamma = np.asarray(gamma, f32)
    beta = np.asarray(beta, f32)
    maps = _host_maps()

    zT = np.ascontiguousarray(z.T)  # [NIN, B]
    WcT = np.ascontiguousarray(Wc.T)  # [NIN, G]
    W1T16 = np.ascontiguousarray(W1.T.astype(np.float16))  # [P, H]
    GS = G // NCORES
    wshT = np.ascontiguousarray(np.asarray(Wshape, f32).T)
    wdrT = np.ascontiguousarray(np.asarray(Wdrop, f32).T)
    bsh = np.asarray(bshape, f32).reshape(1, G)
    bdr = np.asarray(bdrop, f32).reshape(1, G)
    shared = {
        "zT": zT,
        "gamma": np.ascontiguousarray(gamma.reshape(H, 1)),
        "beta": np.ascontiguousarray(beta.reshape(H, 1)),
        "wscT": np.ascontiguousarray(np.asarray(Wscale, f32).T),
        "bsc": np.ascontiguousarray(np.asarray(bscale, f32).reshape(1, G)),
    }

    in_maps = []
    for r in range(NCORES):
        wcT_rot = np.zeros((NIN, G), f32)
        wcT_rot[:, : G - r] = WcT[:, r:]
        bc_rot = np.zeros((1, G), f32)
        bc_rot[0, : G - r] = bc[r:]
        w1t_r = np.zeros((PCPAD, H), np.float16)
        for (ll, lh, gl, gh) in maps[r]:
            w1t_r[ll:lh] = W1T16[gl:gh]
        # partition-major layout: [128, NPT*H], row p holds rows {t*128+p}
        w1t_r = np.ascontiguousarray(
            w1t_r.reshape(NPT, 128, H).transpose(1, 0, 2).reshape(128, NPT * H)
        )
        m = dict(shared)
        m["wcT_rot"] = wcT_rot
        m["bc_rot"] = bc_rot
        m["w1t"] = w1t_r
        m["wshT"] = np.ascontiguousarray(wshT[:, r * GS : (r + 1) * GS])
        m["wdrT"] = np.ascontiguousarray(wdrT[:, r * GS : (r + 1) * GS])
        m["bsh"] = np.ascontiguousarray(bsh[:, r * GS : (r + 1) * GS])
        m["bdr"] = np.ascontiguousarray(bdr[:, r * GS : (r + 1) * GS])
        in_maps.append(m)
    return in_maps


def _run(z, Wc, bc, W1, b1, gamma, beta, Wscale, bscale,
         Wshape, bshape, Wdrop, bdrop, trace=False, trace_cores=None):
    from concourse import bass_utils

    f32 = np.float32
    nc = _build_nc()
    maps = _host_maps()
    in_maps = _prep_in_maps(z, Wc, bc, W1, b1, gamma, beta, Wscale, bscale,
                            Wshape, bshape, Wdrop, bdrop)
    res = bass_utils.run_bass_kernel_spmd(
        nc, in_maps, core_ids=list(range(NCORES)),
        trace=trace, trace_cores=trace_cores,
    )
    outs = res.results

    px_sigma = np.empty((B, P), f32)
    for r in range(NCORES):
        sig_r = np.asarray(outs[r]["px_sig"], f32)
        for (ll, lh, gl, gh) in maps[r]:
            px_sigma[:, gl:gh] = sig_r[:, ll:lh]

    px_shape = np.concatenate([outs[r]["px_shape"] for r in range(NCORES)], axis=1)
    px_scale = outs[0]["px_scale"]
    px_dropout = np.concatenate([outs[r]["px_drop"] for r in range(NCORES)], axis=1)
    return (px_shape, px_scale, px_dropout, px_sigma), res


# revision 46
# speedup vs baseline: 792.7688x; 1.0170x over previous
"""Trainium2 Bass kernel for nn_DecoderVCSC (8-core SPMD).

Reference computation:
    c = z @ Wc.T + bc                                  (B, G)
    px_sigma = exp(-(c[:,iu] - c[:,jv]))               (B, P)   P = G*(G-1)/2
    h = px_sigma @ W1.T + b1                           (B, H)
    BN(training stats) + ReLU
    px_dropout = h @ Wdrop.T + bdrop
    px_shape   = h @ Wshape.T + bshape
    px_scale   = softmax(h @ Wscale.T + bscale)
    returns (px_shape, px_scale, px_dropout, px_sigma)

Sharding: the i<j pair blocks (block i = pairs (i, i+1..G-1)) are dealt
round-robin: core r handles blocks i = 8k + r, k = 0..127.  To keep the
SPMD instruction stream identical across cores, slot k is padded to
L_k = G-1-8k columns on every core (core r's true block length is
G-1-8k-r; the r pad columns are garbage on device and are (a) skipped by
the host unshard and (b) multiplied by host-zeroed W1T rows in the h
contraction).  The per-core shift r is absorbed on the host by passing a
rotated Wc (WcT_rot[:, t] = Wc[t+r, :]), so the device computes
cshift[b, t] = c[b, t+r] with core-independent addressing.

h is accumulated per-core over its p-slice and AllReduced across the 8
cores on-device; BN + heads + softmax run (redundantly) on every core.
b1 provably cancels in train-stats BN and is not sent to the device.
"""

import numpy as np

G = 1024
NIN = 64
H = 128
B = 256
P = G * (G - 1) // 2  # 523776
NCORES = 8
NSLOT = G // NCORES  # 128 slots per core
ALPHA = 1.0
BN_EPS = 1e-3

# Padded per-slot segment lengths/offsets (identical on all cores).
SEG_LEN = [G - 1 - NCORES * k for k in range(NSLOT)]  # 1023 - 8k
_off = np.cumsum([0] + SEG_LEN)
SEG_OFF = [int(x) for x in _off]
PCPAD = SEG_OFF[-1]  # 65920
NPT = PCPAD // 128  # 515 p-tiles of 128
assert PCPAD % 128 == 0

CHUNK = 2048  # sigma free-dim chunk width (elements)
CHUNKS = []
_c0 = 0
while _c0 < PCPAD:
    CHUNKS.append((_c0, min(_c0 + CHUNK, PCPAD)))
    _c0 += CHUNK


def _block_off(i):
    # global p offset of block i: sum_{t<i} (G-1-t)
    return i * (G - 1) - (i * (i - 1)) // 2


def _segments_in(lo, hi):
    """Yield (k, seg_lo, seg_hi) covering [lo,hi) split by slot boundaries."""
    out = []
    for k in range(NSLOT):
        s, e = SEG_OFF[k], SEG_OFF[k + 1]
        a, b = max(s, lo), min(e, hi)
        if a < b:
            out.append((k, a, b))
    return out


_NC_CACHE = {}


def _build_nc(single=False):
    key = "nc1" if single else "nc"
    if key in _NC_CACHE:
        return _NC_CACHE[key]
    import concourse.bass as bass
    import concourse.mybir as mybir
    import concourse.tile as tile
    from concourse import bacc
    from concourse.masks import make_identity

    f32 = mybir.dt.float32
    f32r = mybir.dt.float32r
    f16 = mybir.dt.float16
    AF = mybir.ActivationFunctionType
    ALU = mybir.AluOpType
    AX = mybir.AxisListType

    nc = bacc.Bacc(
        "TRN2", target_bir_lowering=False, debug=False,
        num_devices=1 if single else NCORES,
    )

    _pools = {}

    def _sgl(tc, shape, dtype, name, space="SBUF"):
        pool = _pools["psum" if space == "PSUM" else "const"]
        return pool.tile(shape, dtype, name=name, tag=name)

    # ---- I/O ----
    zT_d = nc.dram_tensor("zT", [NIN, B], f32r, kind="ExternalInput").ap()
    wcT_d = nc.dram_tensor("wcT_rot", [NIN, G], f32r, kind="ExternalInput").ap()
    bc_d = nc.dram_tensor("bc_rot", [1, G], f32r, kind="ExternalInput").ap()
    w1t_d = nc.dram_tensor("w1t", [128, NPT * H], f16, kind="ExternalInput").ap()
    gamma_d = nc.dram_tensor("gamma", [H, 1], f32, kind="ExternalInput").ap()
    beta_d = nc.dram_tensor("beta", [H, 1], f32, kind="ExternalInput").ap()
    GS = G // NCORES  # per-core head-output columns
    wscT_d = nc.dram_tensor("wscT", [H, G], f32r, kind="ExternalInput").ap()
    wshT_d = nc.dram_tensor("wshT", [H, GS], f32r, kind="ExternalInput").ap()
    wdrT_d = nc.dram_tensor("wdrT", [H, GS], f32r, kind="ExternalInput").ap()
    bsc_d = nc.dram_tensor("bsc", [1, G], f32r, kind="ExternalInput").ap()
    bsh_d = nc.dram_tensor("bsh", [1, GS], f32r, kind="ExternalInput").ap()
    bdr_d = nc.dram_tensor("bdr", [1, GS], f32r, kind="ExternalInput").ap()

    psig_d = nc.dram_tensor("px_sig", [B, PCPAD], f16, kind="ExternalOutput").ap()
    psh_d = nc.dram_tensor("px_shape", [B, GS], f32, kind="ExternalOutput").ap()
    psc_d = nc.dram_tensor("px_scale", [B, G], f32, kind="ExternalOutput").ap()
    pdr_d = nc.dram_tensor("px_drop", [B, GS], f32, kind="ExternalOutput").ap()

    from contextlib import ExitStack

    with tile.TileContext(nc) as tc, ExitStack() as _stack:
        _pools["const"] = _stack.enter_context(tc.tile_pool(name="const", bufs=1))
        _pools["psum"] = _stack.enter_context(
            tc.tile_pool(name="constps", bufs=1, space="PSUM")
        )
        # ---------- constants ----------
        identF = _sgl(tc, [128, 128], f32, name="identF")
        make_identity(nc, identF[:])
        ident = _sgl(tc, [128, 128], f16, name="ident")
        nc.vector.tensor_copy(ident[:], identF[:])
        identr = ident[:]
        ones1cf = _sgl(tc, [128, 128], f32, name="ones1cf")
        nc.vector.memset(ones1cf[:], 1.0)
        ones1c = ones1cf[0:1, :]
        ones1f = _sgl(tc, [128, 128], f32r, name="ones1f")
        nc.vector.tensor_copy(ones1f[:], ones1cf[:])
        ones1 = ones1f[0:1, :]

        # ---------- small input loads ----------
        zT_sbf = _sgl(tc, [128, B], f32r, name="zT_sbf")
        nc.sync.dma_start(zT_sbf[0:NIN, :], zT_d[:])
        zT_sb = zT_sbf[0:NIN, :]
        wcT_sbf = _sgl(tc, [128, G], f32r, name="wcT_sbf")
        nc.sync.dma_start(wcT_sbf[0:NIN, :], wcT_d[:])
        wcT_sb = wcT_sbf[0:NIN, :]
        bc_sbf = _sgl(tc, [128, G], f32r, name="bc_sbf")
        nc.sync.dma_start(bc_sbf[0:1, :], bc_d[:])
        bc_sb = bc_sbf[0:1, :]
        gamma_sb = _sgl(tc, [H, 1], f32, name="gamma_sb")
        nc.sync.dma_start(gamma_sb[:], gamma_d[:])
        beta_sb = _sgl(tc, [H, 1], f32, name="beta_sb")
        nc.sync.dma_start(beta_sb[:], beta_d[:])
        hw_sb = _sgl(tc, [H, G + 2 * GS], f32r, name="hw_sb")  # head weights (T)
        nc.sync.dma_start(hw_sb[:, 0:GS], wshT_d[:])
        nc.sync.dma_start(hw_sb[:, GS : 2 * GS], wdrT_d[:])
        nc.sync.dma_start(hw_sb[:, 2 * GS : 2 * GS + G], wscT_d[:])
        hb_sbf = _sgl(tc, [128, G + 2 * GS], f32r, name="hb_sbf")  # head biases
        hb_sb = hb_sbf[0:1, :]
        nc.sync.dma_start(hb_sbf[0:1, 0:GS], bsh_d[:])
        nc.sync.dma_start(hb_sbf[0:1, GS : 2 * GS], bdr_d[:])
        nc.sync.dma_start(hb_sbf[0:1, 2 * GS : 2 * GS + G], bsc_d[:])

        # ---------- cshift = (z @ WcT_rot) + bc_rot,  [2x128 b, 1024 g'] ----------
        ebuf = []  # exp(cshift)
        fbuf = []  # exp(-cshift)
        with tc.tile_pool(name="cpsum", bufs=4, space="PSUM") as cps_pool:
            for bt in range(2):
                e_sb = _sgl(tc, [128, G], f32, name=f"ebuf{bt}")
                f_sb = _sgl(tc, [128, G], f32, name=f"fbuf{bt}")
                for gh in range(2):  # two 512-wide psum tiles
                    cps = cps_pool.tile([128, 512], f32, tag="cps", name="cps")
                    gsl = slice(gh * 512, (gh + 1) * 512)
                    # bias broadcast via rank-1 matmul, then accumulate z@WcT
                    nc.tensor.matmul(
                        cps[:], ones1[:, 0:128], bc_sb[:, gsl], start=True, stop=False
                    )
                    nc.tensor.matmul(
                        cps[:],
                        zT_sb[:, bt * 128 : (bt + 1) * 128],
                        wcT_sb[:, gsl],
                        start=False,
                        stop=True,
                    )
                    nc.scalar.activation(e_sb[:, gsl], cps[:], AF.Exp)
                    nc.scalar.activation(f_sb[:, gsl], cps[:], AF.Exp, scale=-1.0)
                ebuf.append(e_sb)
                fbuf.append(f_sb)

        # ---------- main loop: sigma gen -> HBM + transpose -> h matmul ----------
        h_ps = _sgl(tc, [128, B], f32, name="h_ps", space="PSUM")  # h.T accumulator
        pt_global = 0
        with (
            tc.tile_pool(name="sig", bufs=5) as sig_pool,
            tc.tile_pool(name="sigT", bufs=8) as sigT_pool,
            tc.tile_pool(name="sigTps", bufs=7, space="PSUM") as sigTps_pool,
            tc.tile_pool(name="w1", bufs=12) as w1_pool,
        ):
            for (c0, c1) in CHUNKS:
                W = c1 - c0
                nt = W // 128
                # -- sigma generation (ACT), both b-tiles --
                sig = [
                    sig_pool.tile([128, W], f16, tag=f"sig{bt}", name=f"sig{bt}") for bt in range(2)
                ]
                for (k, a, b) in _segments_in(c0, c1):
                    # sigma[:, a:b] = exp(cshift[:, 8k+1+(a-S_k) : ...] - cshift[:, 8k])
                    j0 = NCORES * k + 1 + (a - SEG_OFF[k])
                    for bt in range(2):
                        nc.vector.tensor_scalar_mul(
                            sig[bt][:, a - c0 : b - c0],
                            ebuf[bt][:, j0 : j0 + (b - a)],
                            fbuf[bt][:, NCORES * k : NCORES * k + 1],
                        )
                # -- write sigma chunk to HBM --
                for bt in range(2):
                    nc.sync.dma_start(
                        psig_d[bt * 128 : (bt + 1) * 128, c0:c1], sig[bt][:]
                    )
                # -- W1T chunk load: [W, H] rows -> SBUF [128, nt*H] --
                w1c = w1_pool.tile([128, nt * H], f16, tag="w1c", name="w1c")
                nc.gpsimd.dma_start(
                    w1c[:], w1t_d[:, (c0 // 128) * H : (c1 // 128) * H]
                )
                # -- per p-tile: transpose sigma, accumulate h --
                for t2 in range(0, nt, 2):
                    npair = min(2, nt - t2)
                    tps = sigTps_pool.tile([128, 512], f16, tag="tps", name="tps")
                    for ti in range(npair):
                        for bt in range(2):
                            nc.tensor.transpose(
                                tps[:, ti * 256 + bt * 128 : ti * 256 + (bt + 1) * 128],
                                sig[bt][:, (t2 + ti) * 128 : (t2 + ti + 1) * 128],
                                identr[:],
                            )
                    sT = sigT_pool.tile([128, 512], f16, tag="sT", name="sT")
                    if (t2 // 2) % 5 == 0:
                        nc.vector.tensor_copy(
                            sT[:, : npair * 256], tps[:, : npair * 256]
                        )
                    else:
                        nc.scalar.copy(
                            sT[:, : npair * 256], tps[:, : npair * 256]
                        )
                    for ti in range(npair):
                        t = t2 + ti
                        nc.tensor.matmul(
                            h_ps[:],
                            w1c[:, t * H : (t + 1) * H],
                            sT[:, ti * 256 : (ti + 1) * 256],
                            start=(pt_global == 0),
                            stop=(pt_global == NPT - 1),
                            skip_group_check=True,
                        )
                        pt_global += 1

        # ---------- AllReduce h across cores ----------
        h_sb = _sgl(tc, [128, B], f32, name="h_sb")
        nc.scalar.copy(h_sb[:], h_ps[:])
        with tc.tile_pool(name="dram", bufs=2, space="DRAM") as dram:
            h_in = dram.tile([128, B], f32, name="hbounce")
            h_out = dram.tile([128, B], f32, name="hbounce")
            nc.sync.dma_start(h_in[:], h_sb[:])
            if single:
                # cost-model variant: stand in for the AllReduce with a copy
                nc.gpsimd.dma_start(h_out[:], h_in[:])
            else:
                nc.gpsimd.collective_compute(
                    "AllReduce",
                    ALU.add,
                    replica_groups=[list(range(NCORES))],
                    ins=[h_in[:].opt()],
                    outs=[h_out[:].opt()],
                )
            hall = _sgl(tc, [128, B], f32, name="hall")
            nc.sync.dma_start(hall[:], h_out[:])

        # ---------- BatchNorm (training stats) + ReLU, in [k, b] layout ----------
        musum = _sgl(tc, [128, 1], f32, name="musum")
        nc.vector.tensor_reduce(musum[:], hall[:], axis=AX.X, op=ALU.add)
        hsq = _sgl(tc, [128, B], f32, name="hsq")
        nc.vector.tensor_tensor(hsq[:], hall[:], hall[:], op=ALU.mult)
        sqsum = _sgl(tc, [128, 1], f32, name="sqsum")
        nc.vector.tensor_reduce(sqsum[:], hsq[:], axis=AX.X, op=ALU.add)
        mu = _sgl(tc, [128, 1], f32, name="mu")
        nc.vector.tensor_scalar_mul(mu[:], musum[:], 1.0 / B)
        musq = _sgl(tc, [128, 1], f32, name="musq")
        nc.vector.tensor_tensor(musq[:], mu[:], mu[:], op=ALU.mult)
        var = _sgl(tc, [128, 1], f32, name="var")
        nc.vector.scalar_tensor_tensor(
            var[:], sqsum[:], 1.0 / B, musq[:],
            op0=ALU.mult, op1=ALU.subtract,
        )
        varp = _sgl(tc, [128, 1], f32, name="varp")
        nc.vector.tensor_scalar_add(varp[:], var[:], BN_EPS)
        i32 = mybir.dt.int32
        magic = _sgl(tc, [128, 1], i32, name="magic")
        nc.vector.memset(magic[:], 0x5F3759DF)
        ihalf = _sgl(tc, [128, 1], i32, name="ihalf")
        nc.vector.tensor_scalar(
            ihalf[:], varp[:].bitcast(i32), 1, None, op0=ALU.arith_shift_right
        )
        yint = _sgl(tc, [128, 1], i32, name="yint")
        nc.vector.tensor_tensor(yint[:], magic[:], ihalf[:], op=ALU.subtract)
        rst = yint[:].bitcast(f32)
        for _it in range(3):
            nt1 = _sgl(tc, [128, 1], f32, name=f"nt1_{_it}")
            nc.vector.tensor_tensor(nt1[:], varp[:], rst, op=ALU.mult)
            nt2 = _sgl(tc, [128, 1], f32, name=f"nt2_{_it}")
            nc.vector.tensor_tensor(nt2[:], nt1[:], rst, op=ALU.mult)
            nt3 = _sgl(tc, [128, 1], f32, name=f"nt3_{_it}")
            nc.vector.tensor_scalar(
                nt3[:], nt2[:], -0.5, 1.5, op0=ALU.mult, op1=ALU.add
            )
            nt4 = _sgl(tc, [128, 1], f32, name=f"nt4_{_it}")
            nc.vector.tensor_tensor(nt4[:], nt3[:], rst, op=ALU.mult)
            rst = nt4[:]
        bnsc = _sgl(tc, [128, 1], f32, name="bnsc")
        nc.vector.tensor_tensor(bnsc[:], gamma_sb[:], rst, op=ALU.mult)
        mbs = _sgl(tc, [128, 1], f32, name="mbs")
        nc.vector.tensor_tensor(mbs[:], mu[:], bnsc[:], op=ALU.mult)
        bnbi = _sgl(tc, [128, 1], f32, name="bnbi")
        nc.vector.tensor_tensor(bnbi[:], beta_sb[:], mbs[:], op=ALU.subtract)
        hbn = _sgl(tc, [128, B], f32r, name="hbn")
        nc.scalar.activation(hbn[:], hall[:], AF.Relu, bias=bnbi[:], scale=bnsc[:])

        # ---------- heads ----------
        # sharded shape/drop: this core computes its GS output columns
        with (
            tc.tile_pool(name="hps", bufs=6, space="PSUM") as hps_pool,
            tc.tile_pool(name="hout", bufs=4) as hout_pool,
        ):
            for hd, (out_d, ev_eng) in enumerate(
                [(psh_d, nc.scalar), (pdr_d, nc.vector)]
            ):
                off = hd * GS
                for bt in range(2):
                    hps = hps_pool.tile([128, 512], f32, tag="hps", name="hps")[:, 0:GS]
                    nc.tensor.matmul(
                        hps[:], ones1[:, 0:128], hb_sb[:, off : off + GS],
                        start=True, stop=False,
                    )
                    nc.tensor.matmul(
                        hps[:],
                        hbn[:, bt * 128 : (bt + 1) * 128],
                        hw_sb[:, off : off + GS],
                        start=False,
                        stop=True,
                    )
                    o_sb = hout_pool.tile([128, GS], f32, tag="osm", name="o_sb")
                    if hd == 0:
                        ev_eng.copy(o_sb[:], hps[:])
                    else:
                        ev_eng.tensor_copy(o_sb[:], hps[:])
                    nc.sync.dma_start(out_d[bt * 128 : (bt + 1) * 128, :], o_sb[:])
            # scale head: full G + softmax, replicated on every core
            for bt in range(2):
                o_sb = hout_pool.tile([128, G], f32, tag="o_sb", name="o_sb")
                for gh in range(2):
                    hps = hps_pool.tile([128, 512], f32, tag="hps", name="hps")
                    gsl = slice(2 * GS + gh * 512, 2 * GS + (gh + 1) * 512)
                    nc.tensor.matmul(
                        hps[:], ones1[:, 0:128], hb_sb[:, gsl],
                        start=True, stop=False,
                    )
                    nc.tensor.matmul(
                        hps[:],
                        hbn[:, bt * 128 : (bt + 1) * 128],
                        hw_sb[:, gsl],
                        start=False,
                        stop=True,
                    )
                    if gh == 0:
                        nc.scalar.copy(o_sb[:, gh * 512 : (gh + 1) * 512], hps[:])
                    else:
                        nc.vector.tensor_copy(
                            o_sb[:, gh * 512 : (gh + 1) * 512], hps[:]
                        )
                mx = hout_pool.tile([128, 1], f32, tag="mx", name="mx")
                nc.vector.tensor_reduce(mx[:], o_sb[:], axis=AX.X, op=ALU.max)
                nmx = hout_pool.tile([128, 1], f32, tag="nmx", name="nmx")
                nc.vector.tensor_scalar_mul(nmx[:], mx[:], -1.0)
                exl = hout_pool.tile([128, G], f32, tag="exl", name="exl")
                nc.scalar.activation(exl[:], o_sb[:], AF.Exp, bias=nmx[:])
                ssum = hout_pool.tile([128, 1], f32, tag="ssum", name="ssum")
                nc.vector.tensor_reduce(ssum[:], exl[:], axis=AX.X, op=ALU.add)
                sinv = hout_pool.tile([128, 1], f32, tag="sinv", name="sinv")
                nc.vector.reciprocal(sinv[:], ssum[:])
                smx = hout_pool.tile([128, G], f32, tag="smx", name="smx")
                nc.vector.tensor_scalar_mul(smx[:], exl[:], sinv[:])
                nc.sync.dma_start(psc_d[bt * 128 : (bt + 1) * 128, :], smx[:])

    nc.compile()
    _NC_CACHE[key] = nc
    return nc


_HOST_CACHE = {}


def _host_maps():
    """Static per-core index maps (depend only on shapes)."""
    if "maps" in _HOST_CACHE:
        return _HOST_CACHE["maps"]
    # per core r, per slot k: true i = 8k+r, valid len = G-1-i, local seg at SEG_OFF[k]
    maps = []
    for r in range(NCORES):
        rows = []  # (local_lo, local_hi, global_lo, global_hi)
        for k in range(NSLOT):
            i = NCORES * k + r
            vlen = G - 1 - i
            if vlen <= 0:
                continue
            o = _block_off(i)
            rows.append((SEG_OFF[k], SEG_OFF[k] + vlen, o, o + vlen))
        maps.append(rows)
    _HOST_CACHE["maps"] = maps
    return maps


def kernel(z, Wc, bc, W1, b1, gamma, beta, Wscale, bscale,
           Wshape, bshape, Wdrop, bdrop):
    return _run(z, Wc, bc, W1, b1, gamma, beta, Wscale, bscale,
                Wshape, bshape, Wdrop, bdrop)[0]


def _prep_in_maps(z, Wc, bc, W1, b1, gamma, beta, Wscale, bscale,
                  Wshape, bshape, Wdrop, bdrop):
    f32 = np.float32
    z = np.asarray(z, f32)
    Wc = np.asarray(Wc, f32)
    bc = np.asarray(bc, f32)
    W1 = np.asarray(W1, f32)
    gamma = np.asarray(gamma, f32)
    beta = np.asarray(beta, f32)
    maps = _host_maps()

    zT = np.ascontiguousarray(z.T)  # [NIN, B]
    WcT = np.ascontiguousarray(Wc.T)  # [NIN, G]
    W1T16 = np.ascontiguousarray(W1.T.astype(np.float16))  # [P, H]
    GS = G // NCORES
    wshT = np.ascontiguousarray(np.asarray(Wshape, f32).T)
    wdrT = np.ascontiguousarray(np.asarray(Wdrop, f32).T)
    bsh = np.asarray(bshape, f32).reshape(1, G)
    bdr = np.asarray(bdrop, f32).reshape(1, G)
    shared = {
        "zT": zT,
        "gamma": np.ascontiguousarray(gamma.reshape(H, 1)),
        "beta": np.ascontiguousarray(beta.reshape(H, 1)),
        "wscT": np.ascontiguousarray(np.asarray(Wscale, f32).T),
        "bsc": np.ascontiguousarray(np.asarray(bscale, f32).reshape(1, G)),
    }

    in_maps = []
    for r in range(NCORES):
        wcT_rot = np.zeros((NIN, G), f32)
        wcT_rot[:, : G - r] = WcT[:, r:]
        bc_rot = np.zeros((1, G), f32)
        bc_rot[0, : G - r] = bc[r:]
        w1t_r = np.zeros((PCPAD, H), np.float16)
        for (ll, lh, gl, gh) in maps[r]:
            w1t_r[ll:lh] = W1T16[gl:gh]
        # partition-major layout: [128, NPT*H], row p holds rows {t*128+p}
        w1t_r = np.ascontiguousarray(
            w1t_r.reshape(NPT, 128, H).transpose(1, 0, 2).reshape(128, NPT * H)
        )
        m = dict(shared)
        m["wcT_rot"] = wcT_rot
        m["bc_rot"] = bc_rot
        m["w1t"] = w1t_r
        m["wshT"] = np.ascontiguousarray(wshT[:, r * GS : (r + 1) * GS])
        m["wdrT"] = np.ascontiguousarray(wdrT[:, r * GS : (r + 1) * GS])
        m["bsh"] = np.ascontiguousarray(bsh[:, r * GS : (r + 1) * GS])
        m["bdr"] = np.ascontiguousarray(bdr[:, r * GS : (r + 1) * GS])
        in_maps.append(m)
    return in_maps


def _run(z, Wc, bc, W1, b1, gamma, beta, Wscale, bscale,
         Wshape, bshape, Wdrop, bdrop, trace=False, trace_cores=None):
    from concourse import bass_utils

    f32 = np.float32
    nc = _build_nc()
    maps = _host_maps()
    in_maps = _prep_in_maps(z, Wc, bc, W1, b1, gamma, beta, Wscale, bscale,
                            Wshape, bshape, Wdrop, bdrop)
    res = bass_utils.run_bass_kernel_spmd(
        nc, in_maps, core_ids=list(range(NCORES)),
        trace=trace, trace_cores=trace_cores,
    )
    outs = res.results

    px_sigma = np.empty((B, P), f32)
    for r in range(NCORES):
        sig_r = np.asarray(outs[r]["px_sig"], f32)
        for (ll, lh, gl, gh) in maps[r]:
            px_sigma[:, gl:gh] = sig_r[:, ll:lh]

    px_shape = np.concatenate([outs[r]["px_shape"] for r in range(NCORES)], axis=1)
    px_scale = outs[0]["px_scale"]
    px_dropout = np.concatenate([outs[r]["px_drop"] for r in range(NCORES)], axis=1)
    return (px_shape, px_scale, px_dropout, px_sigma), res
